# revision 4
# baseline (speedup 1.0000x reference)
"""AttentionGNN (NNConv message-passing GNN) Trainium2 kernel.

Math restructuring: the reference materializes a per-edge [E,H,H] weight
tensor We = reshape(ef @ efW + efb) and does a per-edge GEMV + segment_sum.
We reorder the contraction:

    m[e]   = sum_k ef'[e,k] * (h[src_e] @ efWk)        (ef' = [ef, 1], efWk incl. bias block)
    agg[n] = sum_e[dst=n] m[e] / max(deg[n],1)
           = sum_k (Wk @ (h @ efWk))[n]

where Wk[dst,src] = sum_{e: dst,src} ef'[e,k] / max(deg[dst],1) is a
per-graph weighted adjacency matrix, built once on the host (edge features
do not change across layers).  The device kernel is then pure dense matmul:
no gather/scatter, no [E,H,H] tensor.

Sharding: data-parallel over graphs, 4 graphs per core on 8 cores,
parameters replicated.  Activations live transposed ([H, nodes]) so the
whole layer chains with zero on-device transposes.
"""

import numpy as np
import ml_dtypes

import concourse.bass as bass
import concourse.mybir as mybir
from concourse import bacc
from concourse.tile import TileContext

# ---- problem dimensions (hardcoded per spec) ----
B, L, R, H, EF = 32, 32, 128, 64, 16
NF_IN, EF_IN = 16, 4
LAYERS = 3
NCORES = 8
GPC = B // NCORES          # graphs per core = 4
NL = GPC * L               # lig nodes per core = 128
NR = GPC * R               # rec nodes per core = 512
KC = EF + 1                # 17 adjacency channels (16 edge feats + bias)
SLOPE = 0.01               # leaky_relu negative slope

# matmul operand dtype for the main chain ("bf16" or "f32")
MM_DTYPE = "bf16"

_NP_D = {"bf16": ml_dtypes.bfloat16, "f32": np.float32}
_MY_D = {"bf16": mybir.dt.bfloat16, "f32": mybir.dt.float32}

F32 = mybir.dt.float32
AF = mybir.ActivationFunctionType
ALU = mybir.AluOpType
AX = mybir.AxisListType

_CACHE = {}


# --------------------------------------------------------------------------
# device kernel
# --------------------------------------------------------------------------

def _build_nc(dt_key):
    D = _MY_D[dt_key]
    nc = bacc.Bacc(None, target_bir_lowering=False)

    xl = nc.dram_tensor("xl", [NF_IN, NL], D, kind="ExternalInput")
    xr = nc.dram_tensor("xr", [NF_IN, NR], D, kind="ExternalInput")
    wl = nc.dram_tensor("wl", [NL, KC * NL], D, kind="ExternalInput")
    wr = nc.dram_tensor("wr", [R, GPC * KC * R], D, kind="ExternalInput")
    embw = nc.dram_tensor("embw", [NF_IN, 2 * H], D, kind="ExternalInput")
    hidw = nc.dram_tensor("hidw", [H, 2 * H], D, kind="ExternalInput")
    efwp = nc.dram_tensor("efwp", [H, 2 * LAYERS * KC * H], D, kind="ExternalInput")
    oww = nc.dram_tensor("oww", [H, 2 * LAYERS * H], D, kind="ExternalInput")
    bia = nc.dram_tensor("bia", [H, 16], F32, kind="ExternalInput")

    ops_o = nc.dram_tensor("ops_o", [GPC, L, R], F32, kind="ExternalOutput")
    sc_o = nc.dram_tensor("sc_o", [1, GPC], F32, kind="ExternalOutput")

    NALL = {0: NL, 1: NR}          # type 0 = lig, 1 = rec
    GCHUNKS = [(0, 512), (512, 512), (1024, KC * H - 1024)]

    with TileContext(nc) as tc:
        with (
            tc.tile_pool(name="const", bufs=1) as cpool,
            tc.tile_pool(name="work", bufs=2) as wpool,
            tc.tile_pool(name="hbars", bufs=3) as hpool,
            tc.tile_pool(name="psG", bufs=2, space=bass.MemorySpace.PSUM) as psG,
            tc.tile_pool(name="psA", bufs=1, space=bass.MemorySpace.PSUM) as psA,
        ):
            # ---- load constants/params into SBUF ----
            def load(dram, shape, dtype, tag):
                t = cpool.tile(shape, dtype, tag=tag)
                nc.sync.dma_start(out=t[:], in_=dram[:])
                return t

            xl_sb = load(xl, [NF_IN, NL], D, "xl")
            xr_sb = load(xr, [NF_IN, NR], D, "xr")
            embw_sb = load(embw, [NF_IN, 2 * H], D, "embw")
            hidw_sb = load(hidw, [H, 2 * H], D, "hidw")
            bia_sb = load(bia, [H, 16], F32, "bia")
            efwp_sb = load(efwp, [H, 2 * LAYERS * KC * H], D, "efwp")
            oww_sb = load(oww, [H, 2 * LAYERS * H], D, "oww")
            wl_sb = load(wl, [NL, KC * NL], D, "wl")
            # rec adjacency is the big one: DMA per graph so layer-0 agg of
            # graph 0 can start before the whole 2.2MB lands
            wr_sb = cpool.tile([R, GPC * KC * R], D, tag="wr")
            for g in range(GPC):
                nc.sync.dma_start(
                    out=wr_sb[:, g * KC * R:(g + 1) * KC * R],
                    in_=wr[:, g * KC * R:(g + 1) * KC * R],
                )
            ones_sb = cpool.tile([H, 1], F32, tag="ones")
            nc.vector.memset(ones_sb[:], 1.0)

            def bcol(j):
                return bia_sb[:, j:j + 1]

            # bias columns: 0 lig hid_b, 1 rec hid_b, 2 lig emb_nb,
            # 3 rec emb_nb, 4 + l*4 + t*2 + {0: gb, 1: ob}
            def lrelu(out_ap, in_ap):
                nc.vector.scalar_tensor_tensor(
                    out_ap, in_ap, SLOPE, in_ap, ALU.mult, ALU.max)

            # ---- graph embedding ----
            def embed(t):
                n = NALL[t]
                x_sb = xl_sb if t == 0 else xr_sb
                e1 = psA.tile([H, n], F32, tag="agg")
                nc.tensor.matmul(e1[:], embw_sb[:, t * H:(t + 1) * H], x_sb[:],
                                 start=True, stop=True)
                eb = wpool.tile([H, n], F32, tag=f"xs{t}")
                nc.scalar.activation(eb[:], e1[:], AF.Identity, bias=bcol(2 + t))
                ea = wpool.tile([H, n], D, tag=f"act2{t}")
                lrelu(ea[:], eb[:])
                h1 = psA.tile([H, n], F32, tag="h")
                nc.tensor.matmul(h1[:], hidw_sb[:, t * H:(t + 1) * H], ea[:],
                                 start=True, stop=True)
                hb = hpool.tile([H, n], F32, tag=f"hbar{t}")
                nc.scalar.activation(hb[:], h1[:], AF.Identity, bias=bcol(t))
                return hb

            # ---- one MPNN layer for one graph type ----
            def layer(t, l, hb_in, h0):
                n = NALL[t]
                t2 = l * 2 + t
                act = wpool.tile([H, n], D, tag=f"act{t}")
                lrelu(act[:], hb_in[:])

                aggt = psA.tile([H, n], F32, tag="agg")
                ngr = 1 if t == 0 else GPC
                for g in range(ngr):
                    # G = act_g @ efW'  -> [128 nodes, KC*H]
                    gt = psG.tile([R, KC * H], F32, tag="G")
                    for c0, cn in GCHUNKS:
                        nc.tensor.matmul(
                            gt[:, c0:c0 + cn],
                            act[:, g * R:(g + 1) * R] if t == 1 else act[:],
                            efwp_sb[:, t2 * KC * H + c0: t2 * KC * H + c0 + cn],
                            start=True, stop=True)
                    gs = wpool.tile([R, KC * H], D, tag="Gs")
                    nc.vector.tensor_copy(gs[:, 0:512], gt[:, 0:512])
                    nc.scalar.copy(gs[:, 512:1024], gt[:, 512:1024])
                    nc.vector.tensor_copy(gs[:, 1024:KC * H], gt[:, 1024:KC * H])
                    # aggT_g += G_k^T @ WkT  over k (PSUM accumulation)
                    w_sb = wl_sb if t == 0 else wr_sb
                    base = 0 if t == 0 else g * KC * R
                    on = n if t == 0 else R
                    oc = 0 if t == 0 else g * R
                    for k in range(KC):
                        nc.tensor.matmul(
                            aggt[:, oc:oc + on],
                            gs[:, k * H:(k + 1) * H],
                            w_sb[:, base + k * on: base + (k + 1) * on],
                            start=(k == 0), stop=(k == KC - 1))

                xs = wpool.tile([H, n], F32, tag=f"xs{t}")
                nc.scalar.activation(xs[:], aggt[:], AF.Identity,
                                     bias=bcol(4 + l * 4 + t * 2))
                act2 = wpool.tile([H, n], D, tag=f"act2{t}")
                lrelu(act2[:], xs[:])
                ht = psA.tile([H, n], F32, tag="h")
                nc.tensor.matmul(ht[:], oww_sb[:, t2 * H:(t2 + 1) * H], act2[:],
                                 start=True, stop=True)
                hb = hpool.tile([H, n], F32, tag=f"hbar{t}")
                ob = bcol(4 + l * 4 + t * 2 + 1)
                if h0 is not None:
                    # skip connection: hb = (ht + ob) + h0 in one DVE op
                    nc.vector.scalar_tensor_tensor(
                        hb[:], ht[:], ob, h0[:], ALU.add, ALU.add)
                else:
                    nc.scalar.activation(hb[:], ht[:], AF.Identity, bias=ob)
                return hb

            hbs = [embed(0), embed(1)]
            h0s = [None, None]
            for l in range(LAYERS):
                for t in (0, 1):
                    hbs[t] = layer(t, l, hbs[t],
                                   h0s[t] if l == LAYERS - 1 else None)
                    if l == 0:
                        h0s[t] = hbs[t]

            # ---- per-graph outer products + means (all fp32) ----
            hl, hr = hbs
            opst = psA.tile([L, GPC * R], F32, tag="agg")
            for g in range(GPC):
                nc.tensor.matmul(opst[:, g * R:(g + 1) * R],
                                 hl[:, g * L:(g + 1) * L],
                                 hr[:, g * R:(g + 1) * R],
                                 start=True, stop=True)
            opss = wpool.tile([L, GPC * R], F32, tag="ops")
            nc.vector.tensor_copy(opss[:], opst[:])
            nc.sync.dma_start(out=ops_o[:].rearrange("g l r -> l g r"),
                              in_=opss[:].rearrange("l (g r) -> l g r", g=GPC))

            # mean over (l, r) = dot(sum_l lig, sum_r rec) / (L*R)
            sl = wpool.tile([H, GPC], F32, tag="sl")
            nc.vector.tensor_reduce(
                sl[:], hl[:].rearrange("p (g n) -> p g n", g=GPC),
                axis=AX.X, op=ALU.add)
            sr = wpool.tile([H, GPC], F32, tag="sr")
            nc.vector.tensor_reduce(
                sr[:], hr[:].rearrange("p (g n) -> p g n", g=GPC),
                axis=AX.X, op=ALU.add)
            pr = wpool.tile([H, GPC], F32, tag="pr")
            nc.vector.tensor_mul(pr[:], sl[:], sr[:])
            mm = psA.tile([1, GPC], F32, tag="h")
            nc.tensor.matmul(mm[:], ones_sb[:], pr[:], start=True, stop=True)
            sc = wpool.tile([1, GPC], F32, tag="sc")
            nc.scalar.mul(sc[:], mm[:], 1.0 / (L * R))
            nc.sync.dma_start(out=sc_o[:], in_=sc[:])

    nc.compile()
    return nc


def _get_nc():
    key = ("nc", MM_DTYPE)
    if key not in _CACHE:
        _CACHE[key] = _build_nc(MM_DTYPE)
    return _CACHE[key]


# --------------------------------------------------------------------------
# host-side preprocessing
# --------------------------------------------------------------------------

def _np(x):
    return np.asarray(x, dtype=np.float32)


def _build_adjacency(ef, src, dst, n_per):
    """Per-graph degree-normalized weighted adjacency, [B, src, dst, KC]."""
    src = np.asarray(src, dtype=np.int64)
    dst = np.asarray(dst, dtype=np.int64)
    e = ef.shape[0]
    deg = np.bincount(dst, minlength=B * n_per).astype(np.float32)
    scale = 1.0 / np.maximum(deg, 1.0)
    efp = np.concatenate([ef, np.ones((e, 1), np.float32)], axis=1)
    efp = efp * scale[dst][:, None]
    g = dst // n_per
    sl = src - g * n_per
    dl = dst - g * n_per
    W = np.zeros((B * n_per * n_per, KC), np.float32)
    np.add.at(W, (g * n_per + sl) * n_per + dl, efp)
    return W.reshape(B, n_per, n_per, KC)


def _prepare_inputs(params, lig_x, lig_e, rec_x, rec_e,
                    lig_src, lig_dst, rec_src, rec_dst):
    p = params
    npd = _NP_D[MM_DTYPE]

    ef_l = _np(lig_e) @ _np(p["lig_emb_eW"]) + _np(p["lig_emb_eb"])
    ef_r = _np(rec_e) @ _np(p["rec_emb_eW"]) + _np(p["rec_emb_eb"])
    Wl = _build_adjacency(ef_l, lig_src, lig_dst, L)   # [B, s, d, KC]
    Wr = _build_adjacency(ef_r, rec_src, rec_dst, R)

    embw = np.concatenate([_np(p["lig_emb_nW"]), _np(p["rec_emb_nW"])], axis=1)
    hidw = np.concatenate([_np(p["lig_hid_W"]), _np(p["rec_hid_W"])], axis=1)

    efwp_blocks, oww_blocks = [], []
    bia = np.zeros((H, 16), np.float32)
    bia[:, 0] = _np(p["lig_hid_b"])
    bia[:, 1] = _np(p["rec_hid_b"])
    bia[:, 2] = _np(p["lig_emb_nb"])
    bia[:, 3] = _np(p["rec_emb_nb"])
    for l in range(LAYERS):
        lp = p["layers"][l]
        for t, pre in ((0, "lig_"), (1, "rec_")):
            efw = _np(lp[pre + "efW"]).reshape(EF, H, H).transpose(1, 0, 2)
            efb = _np(lp[pre + "efb"]).reshape(H, H)
            efwp_blocks.append(np.concatenate(
                [efw.reshape(H, EF * H), efb], axis=1))
            oww_blocks.append(_np(lp[pre + "oW"]))
            bia[:, 4 + l * 4 + t * 2] = _np(lp[pre + "gb"])
            bia[:, 4 + l * 4 + t * 2 + 1] = _np(lp[pre + "ob"])
    efwp = np.concatenate(efwp_blocks, axis=1)
    oww = np.concatenate(oww_blocks, axis=1)

    lig_x = _np(lig_x)
    rec_x = _np(rec_x)

    shared = {
        "embw": embw.astype(npd), "hidw": hidw.astype(npd),
        "efwp": efwp.astype(npd), "oww": oww.astype(npd),
        "bia": bia,
    }
    in_maps = []
    for c in range(NCORES):
        gs = slice(c * GPC, (c + 1) * GPC)
        # rec adjacency -> [src, (g_local, k, dst)] contiguous SBUF layout
        wr_c = Wr[gs].transpose(1, 0, 3, 2).reshape(R, GPC * KC * R)
        # lig adjacency -> block-diagonal over the 4 graphs: [128, KC*128]
        wl_c = np.zeros((NL, KC, NL), np.float32)
        Wl_c = Wl[gs]  # [GPC, s, d, KC]
        for gl in range(GPC):
            wl_c[gl * L:(gl + 1) * L, :, gl * L:(gl + 1) * L] = \
                Wl_c[gl].transpose(0, 2, 1)
        in_maps.append({
            "xl": lig_x[c * NL:(c + 1) * NL].T.astype(npd).copy(),
            "xr": rec_x[c * NR:(c + 1) * NR].T.astype(npd).copy(),
            "wl": wl_c.reshape(NL, KC * NL).astype(npd),
            "wr": wr_c.astype(npd),
            **shared,
        })
    return in_maps


def kernel(**inputs):
    nc = _get_nc()
    in_maps = _prepare_inputs(**inputs)
    from concourse.bass_utils import run_bass_kernel_spmd
    res = run_bass_kernel_spmd(nc, in_maps, core_ids=list(range(NCORES)))
    results = res.results
    ops = np.concatenate([r["ops_o"] for r in results], axis=0)
    out = np.concatenate([r["sc_o"].reshape(GPC) for r in results], axis=0)
    return out.astype(np.float32), ops.astype(np.float32)


# --------------------------------------------------------------------------
# reusable jitted runner (for benchmarking without re-compiles)
# --------------------------------------------------------------------------

def _get_runner(in_maps):
    """Returns (run, dev_inputs): `run(*dev_inputs)` executes the kernel on
    all 8 cores and returns (ops_concat, sc_concat). Jit/compile happens once;
    repeated calls are pure dispatch+execute. No donation, so the same device
    buffers can be reused every call."""
    import jax
    from jax.sharding import Mesh, PartitionSpec, NamedSharding
    from jax.experimental.shard_map import shard_map
    from concourse import bass2jax
    from concourse.bass2jax import _bass_exec_p, install_neuronx_cc_hook

    nc = _get_nc()
    install_neuronx_cc_hook()

    in_names, out_names, out_avals, zero_outs = [], [], [], []
    partition_name = (nc.partition_id_tensor.name
                      if nc.partition_id_tensor else None)
    for alloc in nc.m.functions[0].allocations:
        if not isinstance(alloc, mybir.MemoryLocationSet):
            continue
        name = alloc.memorylocations[0].name
        if alloc.kind == "ExternalInput":
            if name != partition_name:
                in_names.append(name)
        elif alloc.kind == "ExternalOutput":
            out_names.append(name)
            shape = tuple(alloc.tensor_shape)
            dtype = mybir.dt.np(alloc.dtype)
            out_avals.append(jax.core.ShapedArray(shape, dtype))
            zero_outs.append(np.zeros(shape, dtype))
    n_params = len(in_names)
    all_names = in_names + out_names
    if partition_name is not None:
        all_names.append(partition_name)

    def _body(*args):
        operands = list(args)
        if partition_name is not None:
            operands.append(bass2jax.partition_id_tensor())
        return tuple(_bass_exec_p.bind(
            *operands,
            out_avals=tuple(out_avals),
            in_names=tuple(all_names),
            out_names=tuple(out_names),
            lowering_input_output_aliases=(),
            sim_require_finite=True,
            sim_require_nnan=True,
            nc=nc,
        ))

    devices = jax.devices()[:NCORES]
    mesh = Mesh(np.asarray(devices), ("core",))
    spec = NamedSharding(mesh, PartitionSpec("core"))
    nio = n_params + len(out_names)
    run = jax.jit(shard_map(
        _body, mesh=mesh, in_specs=(PartitionSpec("core"),) * nio,
        out_specs=(PartitionSpec("core"),) * len(out_names), check_rep=False),
        keep_unused=True)

    concat_in = [
        np.concatenate([np.asarray(in_maps[c][nm]) for c in range(NCORES)],
                       axis=0)
        for nm in in_names
    ]
    concat_zero = [np.zeros((NCORES * z.shape[0], *z.shape[1:]), z.dtype)
                   for z in zero_outs]
    dev_inputs = [jax.device_put(a, spec) for a in concat_in + concat_zero]
    return run, dev_inputs, out_names


# revision 8
# speedup vs baseline: 25.5573x; 25.5573x over previous
"""AttentionGNN (NNConv message-passing GNN) Trainium2 kernel.

Math restructuring: the reference materializes a per-edge [E,H,H] weight
tensor We = reshape(ef @ efW + efb) and does a per-edge GEMV + segment_sum.
We reorder the contraction:

    m[e]   = sum_k ef'[e,k] * (h[src_e] @ efWk)        (ef' = [ef, 1], efWk incl. bias block)
    agg[n] = sum_e[dst=n] m[e] / max(deg[n],1)
           = sum_k (Wk @ (h @ efWk))[n]

where Wk[dst,src] = sum_{e: dst,src} ef'[e,k] / max(deg[dst],1) is a
per-graph weighted adjacency matrix, built once on the host (edge features
do not change across layers).  The device kernel is then pure dense matmul:
no gather/scatter, no [E,H,H] tensor.

Sharding: data-parallel over graphs, 4 graphs per core on 8 cores,
parameters replicated.  Activations live transposed ([H, nodes]) so the
whole layer chains with zero on-device transposes.
"""

import numpy as np
import ml_dtypes

import concourse.bass as bass
import concourse.mybir as mybir
from concourse import bacc
from concourse.tile import TileContext

# ---- problem dimensions (hardcoded per spec) ----
B, L, R, H, EF = 32, 32, 128, 64, 16
NF_IN, EF_IN = 16, 4
LAYERS = 3
NCORES = 8
GPC = B // NCORES          # graphs per core = 4
NL = GPC * L               # lig nodes per core = 128
NR = GPC * R               # rec nodes per core = 512
KC = EF + 1                # 17 adjacency channels (16 edge feats + bias)
SLOPE = 0.01               # leaky_relu negative slope

# matmul operand dtype for the main chain ("bf16" or "f32")
import os
MM_DTYPE = os.environ.get("KMM_DTYPE", "bf16")

_NP_D = {"bf16": ml_dtypes.bfloat16, "f32": np.float32}
_MY_D = {"bf16": mybir.dt.bfloat16, "f32": mybir.dt.float32}

F32 = mybir.dt.float32
AF = mybir.ActivationFunctionType
ALU = mybir.AluOpType
AX = mybir.AxisListType

_CACHE = {}


# --------------------------------------------------------------------------
# device kernel
# --------------------------------------------------------------------------

def _build_nc(dt_key, reps=1, loop_iters=1):
    D = _MY_D[dt_key]
    nc = bacc.Bacc(None, target_bir_lowering=False)

    xl = nc.dram_tensor("xl", [NF_IN, NL], D, kind="ExternalInput")
    xr = nc.dram_tensor("xr", [NF_IN, NR], D, kind="ExternalInput")
    wl = nc.dram_tensor("wl", [NL, KC * NL], D, kind="ExternalInput")
    wr = nc.dram_tensor("wr", [R, GPC * KC * R], D, kind="ExternalInput")
    embw = nc.dram_tensor("embw", [NF_IN, 2 * H], D, kind="ExternalInput")
    hidw = nc.dram_tensor("hidw", [H, 2 * H], D, kind="ExternalInput")
    efwp = nc.dram_tensor("efwp", [H, 2 * LAYERS * KC * H], D, kind="ExternalInput")
    oww = nc.dram_tensor("oww", [H, 2 * LAYERS * H], D, kind="ExternalInput")
    bia = nc.dram_tensor("bia", [H, 16], F32, kind="ExternalInput")

    ops_o = nc.dram_tensor("ops_o", [GPC, L, R], F32, kind="ExternalOutput")
    sc_o = nc.dram_tensor("sc_o", [1, GPC], F32, kind="ExternalOutput")

    NALL = {0: NL, 1: NR}          # type 0 = lig, 1 = rec
    GCHUNKS = [(0, 512), (512, 512), (1024, KC * H - 1024)]

    with TileContext(nc) as tc:
        with (
            tc.tile_pool(name="const", bufs=2 if reps > 1 else 1) as cpool,
            tc.tile_pool(name="work", bufs=2) as wpool,
            tc.tile_pool(name="hbars", bufs=3) as hpool,
            tc.tile_pool(name="psG", bufs=2, space=bass.MemorySpace.PSUM) as psG,
            tc.tile_pool(name="psA", bufs=1, space=bass.MemorySpace.PSUM) as psA,
        ):
            def one_pass():
                # ---- load constants/params into SBUF ----
                def load(dram, shape, dtype, tag):
                    t = cpool.tile(shape, dtype, tag=tag)
                    nc.sync.dma_start(out=t[:], in_=dram[:])
                    return t

                xl_sb = load(xl, [NF_IN, NL], D, "xl")
                xr_sb = load(xr, [NF_IN, NR], D, "xr")
                embw_sb = load(embw, [NF_IN, 2 * H], D, "embw")
                hidw_sb = load(hidw, [H, 2 * H], D, "hidw")
                bia_sb = load(bia, [H, 16], F32, "bia")
                efwp_sb = load(efwp, [H, 2 * LAYERS * KC * H], D, "efwp")
                oww_sb = load(oww, [H, 2 * LAYERS * H], D, "oww")
                wl_sb = load(wl, [NL, KC * NL], D, "wl")
                # rec adjacency is the big one: DMA per graph so layer-0 agg
                # of graph 0 can start before the whole block lands
                wr_sb = cpool.tile([R, GPC * KC * R], D, tag="wr")
                for g in range(GPC):
                    nc.sync.dma_start(
                        out=wr_sb[:, g * KC * R:(g + 1) * KC * R],
                        in_=wr[:, g * KC * R:(g + 1) * KC * R],
                    )
                ones_sb = cpool.tile([H, 1], F32, tag="ones")
                nc.vector.memset(ones_sb[:], 1.0)

                def bcol(j):
                    return bia_sb[:, j:j + 1]

                # bias columns: 0 lig hid_b, 1 rec hid_b, 2 lig emb_nb,
                # 3 rec emb_nb, 4 + l*4 + t*2 + {0: gb, 1: ob}
                def lrelu(out_ap, in_ap):
                    nc.vector.scalar_tensor_tensor(
                        out_ap, in_ap, SLOPE, in_ap, ALU.mult, ALU.max)

                # ---- graph embedding ----
                def embed(t):
                    n = NALL[t]
                    x_sb = xl_sb if t == 0 else xr_sb
                    e1 = psA.tile([H, n], F32, tag="agg")
                    nc.tensor.matmul(e1[:], embw_sb[:, t * H:(t + 1) * H],
                                     x_sb[:], start=True, stop=True)
                    eb = wpool.tile([H, n], F32, tag=f"xs{t}")
                    nc.scalar.activation(eb[:], e1[:], AF.Identity,
                                         bias=bcol(2 + t))
                    ea = wpool.tile([H, n], D, tag=f"act2{t}")
                    lrelu(ea[:], eb[:])
                    h1 = psA.tile([H, n], F32, tag="h")
                    nc.tensor.matmul(h1[:], hidw_sb[:, t * H:(t + 1) * H],
                                     ea[:], start=True, stop=True)
                    hb = hpool.tile([H, n], F32, tag=f"hbar{t}")
                    nc.scalar.activation(hb[:], h1[:], AF.Identity,
                                         bias=bcol(t))
                    return hb

                # ---- one MPNN layer for one graph type ----
                def layer(t, l, hb_in, h0):
                    n = NALL[t]
                    t2 = l * 2 + t
                    act = wpool.tile([H, n], D, tag=f"act{t}")
                    lrelu(act[:], hb_in[:])

                    aggt = psA.tile([H, n], F32, tag="agg")
                    ngr = 1 if t == 0 else GPC
                    for g in range(ngr):
                        # G = act_g @ efW'  -> [128 nodes, KC*H]
                        gt = psG.tile([R, KC * H], F32, tag="G")
                        for c0, cn in GCHUNKS:
                            nc.tensor.matmul(
                                gt[:, c0:c0 + cn],
                                act[:, g * R:(g + 1) * R] if t == 1 else act[:],
                                efwp_sb[:, t2 * KC * H + c0:
                                        t2 * KC * H + c0 + cn],
                                start=True, stop=True)
                        gs = wpool.tile([R, KC * H], D, tag="Gs")
                        nc.vector.tensor_copy(gs[:, 0:512], gt[:, 0:512])
                        nc.scalar.copy(gs[:, 512:1024], gt[:, 512:1024])
                        nc.vector.tensor_copy(gs[:, 1024:KC * H],
                                              gt[:, 1024:KC * H])
                        # aggT_g += G_k^T @ WkT  over k (PSUM accumulation)
                        w_sb = wl_sb if t == 0 else wr_sb
                        base = 0 if t == 0 else g * KC * R
                        on = n if t == 0 else R
                        oc = 0 if t == 0 else g * R
                        for k in range(KC):
                            nc.tensor.matmul(
                                aggt[:, oc:oc + on],
                                gs[:, k * H:(k + 1) * H],
                                w_sb[:, base + k * on: base + (k + 1) * on],
                                start=(k == 0), stop=(k == KC - 1))

                    xs = wpool.tile([H, n], F32, tag=f"xs{t}")
                    nc.scalar.activation(xs[:], aggt[:], AF.Identity,
                                         bias=bcol(4 + l * 4 + t * 2))
                    act2 = wpool.tile([H, n], D, tag=f"act2{t}")
                    lrelu(act2[:], xs[:])
                    ht = psA.tile([H, n], F32, tag="h")
                    nc.tensor.matmul(ht[:], oww_sb[:, t2 * H:(t2 + 1) * H],
                                     act2[:], start=True, stop=True)
                    hb = hpool.tile([H, n], F32, tag=f"hbar{t}")
                    ob = bcol(4 + l * 4 + t * 2 + 1)
                    if h0 is not None:
                        # skip connection: hb = (ht + ob) + h0 in one DVE op
                        nc.vector.scalar_tensor_tensor(
                            hb[:], ht[:], ob, h0[:], ALU.add, ALU.add)
                    else:
                        nc.scalar.activation(hb[:], ht[:], AF.Identity,
                                             bias=ob)
                    return hb

                hbs = [embed(0), embed(1)]
                h0s = [None, None]
                for l in range(LAYERS):
                    for t in (0, 1):
                        hbs[t] = layer(t, l, hbs[t],
                                       h0s[t] if l == LAYERS - 1 else None)
                        if l == 0:
                            h0s[t] = hbs[t]

                # ---- per-graph outer products + means (all fp32) ----
                hl, hr = hbs
                opst = psA.tile([L, GPC * R], F32, tag="agg")
                for g in range(GPC):
                    nc.tensor.matmul(opst[:, g * R:(g + 1) * R],
                                     hl[:, g * L:(g + 1) * L],
                                     hr[:, g * R:(g + 1) * R],
                                     start=True, stop=True)
                opss = wpool.tile([L, GPC * R], F32, tag="ops")
                nc.vector.tensor_copy(opss[:], opst[:])
                nc.sync.dma_start(
                    out=ops_o[:].rearrange("g l r -> l g r"),
                    in_=opss[:].rearrange("l (g r) -> l g r", g=GPC))

                # mean over (l, r) = dot(sum_l lig, sum_r rec) / (L*R)
                sl = wpool.tile([H, GPC], F32, tag="sl")
                nc.vector.tensor_reduce(
                    sl[:], hl[:].rearrange("p (g n) -> p g n", g=GPC),
                    axis=AX.X, op=ALU.add)
                sr = wpool.tile([H, GPC], F32, tag="sr")
                nc.vector.tensor_reduce(
                    sr[:], hr[:].rearrange("p (g n) -> p g n", g=GPC),
                    axis=AX.X, op=ALU.add)
                pr = wpool.tile([H, GPC], F32, tag="pr")
                nc.vector.tensor_mul(pr[:], sl[:], sr[:])
                mm = psA.tile([1, GPC], F32, tag="h")
                nc.tensor.matmul(mm[:], ones_sb[:], pr[:],
                                 start=True, stop=True)
                sc = wpool.tile([1, GPC], F32, tag="sc")
                nc.scalar.mul(sc[:], mm[:], 1.0 / (L * R))
                nc.sync.dma_start(out=sc_o[:], in_=sc[:])

            if loop_iters > 1:
                with tc.For_i(0, loop_iters, 1):
                    for _rep in range(reps):
                        one_pass()
            else:
                for _rep in range(reps):
                    one_pass()

    nc.compile()
    return nc


def _get_nc(reps=1, loop_iters=1):
    key = ("nc", MM_DTYPE, reps, loop_iters)
    if key not in _CACHE:
        _CACHE[key] = _build_nc(MM_DTYPE, reps, loop_iters)
    return _CACHE[key]


# --------------------------------------------------------------------------
# host-side preprocessing
# --------------------------------------------------------------------------

def _np(x):
    return np.asarray(x, dtype=np.float32)


def _build_adjacency(ef, src, dst, n_per):
    """Per-graph degree-normalized weighted adjacency, [B, src, dst, KC]."""
    src = np.asarray(src, dtype=np.int64)
    dst = np.asarray(dst, dtype=np.int64)
    e = ef.shape[0]
    deg = np.bincount(dst, minlength=B * n_per).astype(np.float32)
    scale = 1.0 / np.maximum(deg, 1.0)
    efp = np.concatenate([ef, np.ones((e, 1), np.float32)], axis=1)
    efp = efp * scale[dst][:, None]
    g = dst // n_per
    sl = src - g * n_per
    dl = dst - g * n_per
    ok = (sl >= 0) & (sl < n_per)   # edges never cross graphs per spec
    W = np.zeros((B * n_per * n_per, KC), np.float32)
    np.add.at(W, ((g * n_per + sl) * n_per + dl)[ok], efp[ok])
    return W.reshape(B, n_per, n_per, KC)


def _prepare_inputs(params, lig_x, lig_e, rec_x, rec_e,
                    lig_src, lig_dst, rec_src, rec_dst):
    p = params
    npd = _NP_D[MM_DTYPE]

    ef_l = _np(lig_e) @ _np(p["lig_emb_eW"]) + _np(p["lig_emb_eb"])
    ef_r = _np(rec_e) @ _np(p["rec_emb_eW"]) + _np(p["rec_emb_eb"])
    Wl = _build_adjacency(ef_l, lig_src, lig_dst, L)   # [B, s, d, KC]
    Wr = _build_adjacency(ef_r, rec_src, rec_dst, R)

    embw = np.concatenate([_np(p["lig_emb_nW"]), _np(p["rec_emb_nW"])], axis=1)
    hidw = np.concatenate([_np(p["lig_hid_W"]), _np(p["rec_hid_W"])], axis=1)

    efwp_blocks, oww_blocks = [], []
    bia = np.zeros((H, 16), np.float32)
    bia[:, 0] = _np(p["lig_hid_b"])
    bia[:, 1] = _np(p["rec_hid_b"])
    bia[:, 2] = _np(p["lig_emb_nb"])
    bia[:, 3] = _np(p["rec_emb_nb"])
    for l in range(LAYERS):
        lp = p["layers"][l]
        for t, pre in ((0, "lig_"), (1, "rec_")):
            efw = _np(lp[pre + "efW"]).reshape(EF, H, H).transpose(1, 0, 2)
            efb = _np(lp[pre + "efb"]).reshape(H, H)
            efwp_blocks.append(np.concatenate(
                [efw.reshape(H, EF * H), efb], axis=1))
            oww_blocks.append(_np(lp[pre + "oW"]))
            bia[:, 4 + l * 4 + t * 2] = _np(lp[pre + "gb"])
            bia[:, 4 + l * 4 + t * 2 + 1] = _np(lp[pre + "ob"])
    efwp = np.concatenate(efwp_blocks, axis=1)
    oww = np.concatenate(oww_blocks, axis=1)

    lig_x = _np(lig_x)
    rec_x = _np(rec_x)

    shared = {
        "embw": embw.astype(npd), "hidw": hidw.astype(npd),
        "efwp": efwp.astype(npd), "oww": oww.astype(npd),
        "bia": bia,
    }
    in_maps = []
    for c in range(NCORES):
        gs = slice(c * GPC, (c + 1) * GPC)
        # rec adjacency -> [src, (g_local, k, dst)] contiguous SBUF layout
        wr_c = Wr[gs].transpose(1, 0, 3, 2).reshape(R, GPC * KC * R)
        # lig adjacency -> block-diagonal over the 4 graphs: [128, KC*128]
        wl_c = np.zeros((NL, KC, NL), np.float32)
        Wl_c = Wl[gs]  # [GPC, s, d, KC]
        for gl in range(GPC):
            wl_c[gl * L:(gl + 1) * L, :, gl * L:(gl + 1) * L] = \
                Wl_c[gl].transpose(0, 2, 1)
        in_maps.append({
            "xl": lig_x[c * NL:(c + 1) * NL].T.astype(npd).copy(),
            "xr": rec_x[c * NR:(c + 1) * NR].T.astype(npd).copy(),
            "wl": wl_c.reshape(NL, KC * NL).astype(npd),
            "wr": wr_c.astype(npd),
            **shared,
        })
    return in_maps


def kernel(**inputs):
    nc = _get_nc()
    in_maps = _prepare_inputs(**inputs)
    from concourse.bass_utils import run_bass_kernel_spmd
    res = run_bass_kernel_spmd(nc, in_maps, core_ids=list(range(NCORES)))
    results = res.results
    ops = np.concatenate([r["ops_o"] for r in results], axis=0)
    out = np.concatenate([r["sc_o"].reshape(GPC) for r in results], axis=0)
    return out.astype(np.float32), ops.astype(np.float32)


# --------------------------------------------------------------------------
# reusable jitted runner (for benchmarking without re-compiles)
# --------------------------------------------------------------------------

def _get_runner(in_maps, reps=1, loop_iters=1):
    """Returns (run, dev_inputs): `run(*dev_inputs)` executes the kernel on
    all 8 cores and returns (ops_concat, sc_concat). Jit/compile happens once;
    repeated calls are pure dispatch+execute. No donation, so the same device
    buffers can be reused every call."""
    import jax
    from jax.sharding import Mesh, PartitionSpec, NamedSharding
    from jax.experimental.shard_map import shard_map
    from concourse import bass2jax
    from concourse.bass2jax import _bass_exec_p, install_neuronx_cc_hook

    nc = _get_nc(reps, loop_iters)
    install_neuronx_cc_hook()

    in_names, out_names, out_avals, zero_outs = [], [], [], []
    partition_name = (nc.partition_id_tensor.name
                      if nc.partition_id_tensor else None)
    for alloc in nc.m.functions[0].allocations:
        if not isinstance(alloc, mybir.MemoryLocationSet):
            continue
        name = alloc.memorylocations[0].name
        if alloc.kind == "ExternalInput":
            if name != partition_name:
                in_names.append(name)
        elif alloc.kind == "ExternalOutput":
            out_names.append(name)
            shape = tuple(alloc.tensor_shape)
            dtype = mybir.dt.np(alloc.dtype)
            out_avals.append(jax.core.ShapedArray(shape, dtype))
            zero_outs.append(np.zeros(shape, dtype))
    n_params = len(in_names)
    all_names = in_names + out_names
    if partition_name is not None:
        all_names.append(partition_name)

    def _body(*args):
        operands = list(args)
        if partition_name is not None:
            operands.append(bass2jax.partition_id_tensor())
        return tuple(_bass_exec_p.bind(
            *operands,
            out_avals=tuple(out_avals),
            in_names=tuple(all_names),
            out_names=tuple(out_names),
            lowering_input_output_aliases=(),
            sim_require_finite=True,
            sim_require_nnan=True,
            nc=nc,
        ))

    devices = jax.devices()[:NCORES]
    mesh = Mesh(np.asarray(devices), ("core",))
    spec = NamedSharding(mesh, PartitionSpec("core"))
    nio = n_params + len(out_names)
    run = jax.jit(shard_map(
        _body, mesh=mesh, in_specs=(PartitionSpec("core"),) * nio,
        out_specs=(PartitionSpec("core"),) * len(out_names), check_rep=False),
        keep_unused=True)

    concat_in = [
        np.concatenate([np.asarray(in_maps[c][nm]) for c in range(NCORES)],
                       axis=0)
        for nm in in_names
    ]
    concat_zero = [np.zeros((NCORES * z.shape[0], *z.shape[1:]), z.dtype)
                   for z in zero_outs]
    dev_inputs = [jax.device_put(a, spec) for a in concat_in + concat_zero]
    return run, dev_inputs, out_names


# revision 14
# speedup vs baseline: 47.0982x; 1.8428x over previous
"""AttentionGNN (NNConv message-passing GNN) Trainium2 kernel.

Math restructuring: the reference materializes a per-edge [E,H,H] weight
tensor We = reshape(ef @ efW + efb) and does a per-edge GEMV + segment_sum.
We reorder the contraction:

    m[e]   = sum_k ef'[e,k] * (h[src_e] @ efWk)        (ef' = [ef, 1], efWk incl. bias block)
    agg[n] = sum_e[dst=n] m[e] / max(deg[n],1)
           = sum_k (Wk @ (h @ efWk))[n]

where Wk[dst,src] = sum_{e: dst,src} ef'[e,k] / max(deg[dst],1) is a
per-graph weighted adjacency matrix, built once on the host (edge features
do not change across layers).  The device kernel is then pure dense matmul:
no gather/scatter, no [E,H,H] tensor.

Sharding: data-parallel over graphs, 4 graphs per core on 8 cores,
parameters replicated.  Activations live transposed ([H, nodes]) so the
whole layer chains with zero on-device transposes.
"""

import numpy as np
import ml_dtypes

import concourse.bass as bass
import concourse.mybir as mybir
from concourse import bacc
from concourse.tile import TileContext

# ---- problem dimensions (hardcoded per spec) ----
B, L, R, H, EF = 32, 32, 128, 64, 16
NF_IN, EF_IN = 16, 4
LAYERS = 3
NCORES = 8
GPC = B // NCORES          # graphs per core = 4
NL = GPC * L               # lig nodes per core = 128
NR = GPC * R               # rec nodes per core = 512
KC = EF + 1                # 17 adjacency channels (16 edge feats + bias)
SLOPE = 0.01               # leaky_relu negative slope

# matmul operand dtype for the main chain ("bf16" or "f32")
import os
MM_DTYPE = os.environ.get("KMM_DTYPE", "bf16")

_NP_D = {"bf16": ml_dtypes.bfloat16, "f16": np.float16, "f32": np.float32}
_MY_D = {"bf16": mybir.dt.bfloat16, "f16": mybir.dt.float16,
         "f32": mybir.dt.float32}

F32 = mybir.dt.float32
AF = mybir.ActivationFunctionType
ALU = mybir.AluOpType
AX = mybir.AxisListType

_CACHE = {}
LRELU_ACT = True   # use HW Lrelu activation (not implemented in CoreSim)


# --------------------------------------------------------------------------
# device kernel
# --------------------------------------------------------------------------

def _build_nc(dt_key, reps=1, loop_iters=1):
    D = _MY_D[dt_key]
    lrelu_act = LRELU_ACT
    nc = bacc.Bacc(None, target_bir_lowering=False)

    xl = nc.dram_tensor("xl", [NF_IN, NL], D, kind="ExternalInput")
    xr = nc.dram_tensor("xr", [NF_IN, NR], D, kind="ExternalInput")
    wl = nc.dram_tensor("wl", [NL, KC * NL], D, kind="ExternalInput")
    wr = nc.dram_tensor("wr", [R, GPC * KC * R], D, kind="ExternalInput")
    embw = nc.dram_tensor("embw", [NF_IN, 2 * H], D, kind="ExternalInput")
    hidw = nc.dram_tensor("hidw", [H, 2 * H], D, kind="ExternalInput")
    efwp = nc.dram_tensor("efwp", [H, 2 * LAYERS * KC * H], D, kind="ExternalInput")
    oww = nc.dram_tensor("oww", [H, 2 * LAYERS * H], D, kind="ExternalInput")
    bia = nc.dram_tensor("bia", [H, 16], F32, kind="ExternalInput")

    ops_o = nc.dram_tensor("ops_o", [GPC, L, R], F32, kind="ExternalOutput")
    sc_o = nc.dram_tensor("sc_o", [1, GPC], F32, kind="ExternalOutput")

    NALL = {0: NL, 1: NR}          # type 0 = lig, 1 = rec

    with TileContext(nc) as tc:
        with (
            tc.tile_pool(name="const", bufs=2 if reps * loop_iters > 1 else 1)
                as cpool,
            tc.tile_pool(name="work", bufs=3) as wpool,
            tc.tile_pool(name="hbars", bufs=3) as hpool,
            # PSUM budget (8 banks): Ga 2x2 + Gb 1 + agg0 + agg1 + h = 8
            tc.tile_pool(name="psG", bufs=2, space=bass.MemorySpace.PSUM) as psG,
            tc.tile_pool(name="psGb", bufs=1, space=bass.MemorySpace.PSUM) as psGb,
            tc.tile_pool(name="psA", bufs=1, space=bass.MemorySpace.PSUM) as psA,
            tc.tile_pool(name="psH", bufs=1, space=bass.MemorySpace.PSUM) as psH,
        ):
            def one_pass():
                # ---- load constants/params into SBUF ----
                # split across both HWDGE queues (SP + Activation)
                def load(dram, shape, dtype, tag, eng):
                    t = cpool.tile(shape, dtype, tag=tag)
                    eng.dma_start(out=t[:], in_=dram[:])
                    return t

                # small consts on the (otherwise idle) Pool SWDGE queue
                bia_sb = load(bia, [H, 16], F32, "bia", nc.gpsimd)
                xl_sb = load(xl, [NF_IN, NL], D, "xl", nc.gpsimd)
                xr_sb = load(xr, [NF_IN, NR], D, "xr", nc.gpsimd)
                embw_sb = load(embw, [NF_IN, 2 * H], D, "embw", nc.gpsimd)
                hidw_sb = load(hidw, [H, 2 * H], D, "hidw", nc.gpsimd)
                oww_sb = load(oww, [H, 2 * LAYERS * H], D, "oww", nc.gpsimd)
                # big blocks on the SP HWDGE queue, interleaved by need-time
                efwp_sb = cpool.tile([H, 2 * LAYERS * KC * H], D, tag="efwp")
                wl_sb = cpool.tile([NL, KC * NL], D, tag="wl")
                wr_sb = cpool.tile([R, GPC * KC * R], D, tag="wr")
                CW = 2 * KC * H
                def efwp_chunk(l):
                    nc.sync.dma_start(out=efwp_sb[:, l * CW:(l + 1) * CW],
                                      in_=efwp[:, l * CW:(l + 1) * CW])
                def wr_chunk(g):
                    nc.sync.dma_start(
                        out=wr_sb[:, g * KC * R:(g + 1) * KC * R],
                        in_=wr[:, g * KC * R:(g + 1) * KC * R])
                efwp_chunk(0)
                nc.sync.dma_start(out=wl_sb[:], in_=wl[:])
                wr_chunk(0)
                efwp_chunk(1)
                wr_chunk(1)
                efwp_chunk(2)
                wr_chunk(2)
                wr_chunk(3)
                ones_sb = cpool.tile([H, 1], F32, tag="ones")
                nc.gpsimd.memset(ones_sb[:], 1.0)

                def bcol(j):
                    return bia_sb[:, j:j + 1]

                # bias columns: 0 lig hid_b, 1 rec hid_b, 2 lig emb_nb,
                # 3 rec emb_nb, 4 + l*4 + t*2 + {0: gb, 1: ob}
                def stt_lrelu(out_ap, in_ap):
                    nc.vector.scalar_tensor_tensor(
                        out_ap, in_ap, SLOPE, in_ap, ALU.mult, ALU.max)

                def act_lrelu_bias(out_ap, psum_ap, bias_ap, t):
                    """out = leaky_relu(psum + bias), PSUM -> SBUF."""
                    if lrelu_act:
                        nc.scalar.activation(out_ap, psum_ap, AF.Lrelu,
                                             bias=bias_ap, alpha=SLOPE)
                    else:  # sim fallback (CoreSim lacks Lrelu)
                        n = psum_ap.shape[-1]
                        tmp = wpool.tile([H, n], F32, tag=f"xs{t}")
                        nc.scalar.activation(tmp[:], psum_ap, AF.Identity,
                                             bias=bias_ap)
                        stt_lrelu(out_ap, tmp[:])

                # ---- graph embedding: returns h1 PSUM (pending hid_b) ----
                def embed(t):
                    n = NALL[t]
                    x_sb = xl_sb if t == 0 else xr_sb
                    e1 = psA.tile([H, n], F32, tag=f"agg{t}")
                    nc.tensor.matmul(e1[:], embw_sb[:, t * H:(t + 1) * H],
                                     x_sb[:], start=True, stop=True)
                    ea = wpool.tile([H, n], D, tag=f"act2{t}")
                    act_lrelu_bias(ea[:], e1[:], bcol(2 + t), t)
                    h1 = psH.tile([H, n], F32, tag="h")
                    nc.tensor.matmul(h1[:], hidw_sb[:, t * H:(t + 1) * H],
                                     ea[:], start=True, stop=True)
                    return h1

                # ---- one MPNN layer for one graph type ----
                # src: ("psum", ht, bias_col_idx) or ("sbuf", hb)
                def layer(t, l, src, h0):
                    n = NALL[t]
                    t2 = l * 2 + t
                    act = wpool.tile([H, n], D, tag=f"act{t}")
                    if src[0] == "psum":
                        act_lrelu_bias(act[:], src[1][:], bcol(src[2]), t)
                    else:
                        stt_lrelu(act[:], src[1][:])

                    aggt = psA.tile([H, n], F32, tag=f"agg{t}")
                    ngr = 1 if t == 0 else GPC
                    base_c = t2 * KC * H

                    def lhs_g(g):
                        return act[:, g * R:(g + 1) * R] if t == 1 else act[:]

                    # bias-block G for all graphs up front: one bank, one copy
                    gball = psGb.tile([R, ngr * H], F32, tag="Gb")
                    for g in range(ngr):
                        nc.tensor.matmul(
                            gball[:, g * H:(g + 1) * H], lhs_g(g),
                            efwp_sb[:, base_c + 1024: base_c + KC * H],
                            start=True, stop=True)
                    gsb = wpool.tile([R, ngr * H], D, tag=f"gsb{t}")
                    nc.vector.tensor_copy(gsb[:], gball[:])

                    for g in range(ngr):
                        # G = act_g @ efW' (first 16 blocks) -> [128, 1024]
                        ga = psG.tile([R, 1024], F32, tag="Ga")
                        for c0 in (0, 512):
                            nc.tensor.matmul(
                                ga[:, c0:c0 + 512], lhs_g(g),
                                efwp_sb[:, base_c + c0: base_c + c0 + 512],
                                start=True, stop=True)
                        gs = wpool.tile([R, 1024], D,
                                        tag="Gs" if t == 1 else "Gsl")
                        nc.vector.tensor_copy(gs[:, 0:576], ga[:, 0:576])
                        nc.scalar.copy(gs[:, 576:1024], ga[:, 576:1024])
                        # aggT_g += G_k^T @ WkT, bias channel first
                        w_sb = wl_sb if t == 0 else wr_sb
                        base = 0 if t == 0 else g * KC * R
                        on = n if t == 0 else R
                        oc = 0 if t == 0 else g * R
                        nc.tensor.matmul(
                            aggt[:, oc:oc + on],
                            gsb[:, g * H:(g + 1) * H],
                            w_sb[:, base + (KC - 1) * on: base + KC * on],
                            start=True, stop=False)
                        for k in range(KC - 1):
                            nc.tensor.matmul(
                                aggt[:, oc:oc + on],
                                gs[:, k * H:(k + 1) * H],
                                w_sb[:, base + k * on: base + (k + 1) * on],
                                start=False, stop=(k == KC - 2))

                    act2 = wpool.tile([H, n], D, tag=f"act2{t}")
                    act_lrelu_bias(act2[:], aggt[:], bcol(4 + l * 4 + t * 2), t)
                    ht = psH.tile([H, n], F32, tag="h")
                    nc.tensor.matmul(ht[:], oww_sb[:, t2 * H:(t2 + 1) * H],
                                     act2[:], start=True, stop=True)
                    ob = 4 + l * 4 + t * 2 + 1
                    if l == 0:
                        # keep layer-0 output in SBUF f32: skip-conn source
                        hb = hpool.tile([H, n], F32, tag=f"hbar{t}")
                        nc.scalar.activation(hb[:], ht[:], AF.Identity,
                                             bias=bcol(ob))
                        return ("sbuf", hb), hb
                    if l == 1:
                        return ("psum", ht, ob), None
                    # l == 2: final = (ht + ob) + h0  in one DVE op
                    hb = hpool.tile([H, n], F32, tag=f"hbar{t}")
                    if t == 1:  # per-graph chunks let outer products start
                        for g in range(GPC):
                            s = slice(g * R, (g + 1) * R)
                            nc.vector.scalar_tensor_tensor(
                                hb[:, s], ht[:, s], bcol(ob), h0[:, s],
                                ALU.add, ALU.add)
                    else:
                        nc.vector.scalar_tensor_tensor(
                            hb[:], ht[:], bcol(ob), h0[:], ALU.add, ALU.add)
                    return ("sbuf", hb), None

                srcs = []
                for t in (0, 1):
                    srcs.append(("psum", embed(t), t))  # pending hid_b bias
                h0s = [None, None]
                for l in range(LAYERS):
                    for t in (0, 1):
                        srcs[t], maybe_h0 = layer(t, l, srcs[t], h0s[t])
                        if l == 0:
                            h0s[t] = maybe_h0
                finals = [srcs[0][1], srcs[1][1]]

                # ---- per-graph outer products + means (all fp32) ----
                hl, hr = finals
                sl = wpool.tile([H, GPC], F32, tag="sl")
                nc.vector.tensor_reduce(
                    sl[:], hl[:].rearrange("p (g n) -> p g n", g=GPC),
                    axis=AX.X, op=ALU.add)
                sr = wpool.tile([H, GPC], F32, tag="sr")
                nc.vector.tensor_reduce(
                    sr[:], hr[:].rearrange("p (g n) -> p g n", g=GPC),
                    axis=AX.X, op=ALU.add)
                pr = wpool.tile([H, GPC], F32, tag="pr")
                nc.vector.tensor_mul(pr[:], sl[:], sr[:])
                mm = psH.tile([1, GPC], F32, tag="h")
                nc.tensor.matmul(mm[:], ones_sb[:], pr[:],
                                 start=True, stop=True)
                sc = wpool.tile([1, GPC], F32, tag="sc")
                nc.scalar.mul(sc[:], mm[:], 1.0 / (L * R))
                nc.sync.dma_start(out=sc_o[:], in_=sc[:])
                opst = psA.tile([L, GPC * R], F32, tag="agg1")
                for g in range(GPC):
                    nc.tensor.matmul(opst[:, g * R:(g + 1) * R],
                                     hl[:, g * L:(g + 1) * L],
                                     hr[:, g * R:(g + 1) * R],
                                     start=True, stop=True)
                opss = wpool.tile([L, GPC * R], F32, tag="ops")
                nc.vector.tensor_copy(opss[:], opst[:])
                nc.sync.dma_start(
                    out=ops_o[:].rearrange("g l r -> l g r"),
                    in_=opss[:].rearrange("l (g r) -> l g r", g=GPC))


            if loop_iters > 1:
                with tc.For_i(0, loop_iters, 1):
                    for _rep in range(reps):
                        one_pass()
            else:
                for _rep in range(reps):
                    one_pass()

    nc.compile()
    return nc


def _get_nc(reps=1, loop_iters=1):
    key = ("nc", MM_DTYPE, reps, loop_iters, LRELU_ACT)
    if key not in _CACHE:
        _CACHE[key] = _build_nc(MM_DTYPE, reps, loop_iters)
    return _CACHE[key]


# --------------------------------------------------------------------------
# host-side preprocessing
# --------------------------------------------------------------------------

def _np(x):
    return np.asarray(x, dtype=np.float32)


def _build_adjacency(ef, src, dst, n_per):
    """Per-graph degree-normalized weighted adjacency, [B, src, dst, KC]."""
    src = np.asarray(src, dtype=np.int64)
    dst = np.asarray(dst, dtype=np.int64)
    e = ef.shape[0]
    deg = np.bincount(dst, minlength=B * n_per).astype(np.float32)
    scale = 1.0 / np.maximum(deg, 1.0)
    efp = np.concatenate([ef, np.ones((e, 1), np.float32)], axis=1)
    efp = efp * scale[dst][:, None]
    g = dst // n_per
    sl = src - g * n_per
    dl = dst - g * n_per
    ok = (sl >= 0) & (sl < n_per)   # edges never cross graphs per spec
    W = np.zeros((B * n_per * n_per, KC), np.float32)
    np.add.at(W, ((g * n_per + sl) * n_per + dl)[ok], efp[ok])
    return W.reshape(B, n_per, n_per, KC)


def _prepare_inputs(params, lig_x, lig_e, rec_x, rec_e,
                    lig_src, lig_dst, rec_src, rec_dst):
    p = params
    npd = _NP_D[MM_DTYPE]

    ef_l = _np(lig_e) @ _np(p["lig_emb_eW"]) + _np(p["lig_emb_eb"])
    ef_r = _np(rec_e) @ _np(p["rec_emb_eW"]) + _np(p["rec_emb_eb"])
    Wl = _build_adjacency(ef_l, lig_src, lig_dst, L)   # [B, s, d, KC]
    Wr = _build_adjacency(ef_r, rec_src, rec_dst, R)

    embw = np.concatenate([_np(p["lig_emb_nW"]), _np(p["rec_emb_nW"])], axis=1)
    hidw = np.concatenate([_np(p["lig_hid_W"]), _np(p["rec_hid_W"])], axis=1)

    efwp_blocks, oww_blocks = [], []
    bia = np.zeros((H, 16), np.float32)
    bia[:, 0] = _np(p["lig_hid_b"])
    bia[:, 1] = _np(p["rec_hid_b"])
    bia[:, 2] = _np(p["lig_emb_nb"])
    bia[:, 3] = _np(p["rec_emb_nb"])
    for l in range(LAYERS):
        lp = p["layers"][l]
        for t, pre in ((0, "lig_"), (1, "rec_")):
            efw = _np(lp[pre + "efW"]).reshape(EF, H, H).transpose(1, 0, 2)
            efb = _np(lp[pre + "efb"]).reshape(H, H)
            efwp_blocks.append(np.concatenate(
                [efw.reshape(H, EF * H), efb], axis=1))
            oww_blocks.append(_np(lp[pre + "oW"]))
            bia[:, 4 + l * 4 + t * 2] = _np(lp[pre + "gb"])
            bia[:, 4 + l * 4 + t * 2 + 1] = _np(lp[pre + "ob"])
    efwp = np.concatenate(efwp_blocks, axis=1)
    oww = np.concatenate(oww_blocks, axis=1)

    lig_x = _np(lig_x)
    rec_x = _np(rec_x)

    shared = {
        "embw": embw.astype(npd), "hidw": hidw.astype(npd),
        "efwp": efwp.astype(npd), "oww": oww.astype(npd),
        "bia": bia,
    }
    in_maps = []
    for c in range(NCORES):
        gs = slice(c * GPC, (c + 1) * GPC)
        # rec adjacency -> [src, (g_local, k, dst)] contiguous SBUF layout
        wr_c = Wr[gs].transpose(1, 0, 3, 2).reshape(R, GPC * KC * R)
        # lig adjacency -> block-diagonal over the 4 graphs: [128, KC*128]
        wl_c = np.zeros((NL, KC, NL), np.float32)
        Wl_c = Wl[gs]  # [GPC, s, d, KC]
        for gl in range(GPC):
            wl_c[gl * L:(gl + 1) * L, :, gl * L:(gl + 1) * L] = \
                Wl_c[gl].transpose(0, 2, 1)
        in_maps.append({
            "xl": lig_x[c * NL:(c + 1) * NL].T.astype(npd).copy(),
            "xr": rec_x[c * NR:(c + 1) * NR].T.astype(npd).copy(),
            "wl": wl_c.reshape(NL, KC * NL).astype(npd),
            "wr": wr_c.astype(npd),
            **shared,
        })
    return in_maps


def kernel(**inputs):
    nc = _get_nc()
    in_maps = _prepare_inputs(**inputs)
    from concourse.bass_utils import run_bass_kernel_spmd
    res = run_bass_kernel_spmd(nc, in_maps, core_ids=list(range(NCORES)))
    results = res.results
    ops = np.concatenate([r["ops_o"] for r in results], axis=0)
    out = np.concatenate([r["sc_o"].reshape(GPC) for r in results], axis=0)
    return out.astype(np.float32), ops.astype(np.float32)


# --------------------------------------------------------------------------
# reusable jitted runner (for benchmarking without re-compiles)
# --------------------------------------------------------------------------

def _get_runner(in_maps, reps=1, loop_iters=1):
    """Returns (run, dev_inputs): `run(*dev_inputs)` executes the kernel on
    all 8 cores and returns (ops_concat, sc_concat). Jit/compile happens once;
    repeated calls are pure dispatch+execute. No donation, so the same device
    buffers can be reused every call."""
    import jax
    from jax.sharding import Mesh, PartitionSpec, NamedSharding
    from jax.experimental.shard_map import shard_map
    from concourse import bass2jax
    from concourse.bass2jax import _bass_exec_p, install_neuronx_cc_hook

    nc = _get_nc(reps, loop_iters)
    install_neuronx_cc_hook()

    in_names, out_names, out_avals, zero_outs = [], [], [], []
    partition_name = (nc.partition_id_tensor.name
                      if nc.partition_id_tensor else None)
    for alloc in nc.m.functions[0].allocations:
        if not isinstance(alloc, mybir.MemoryLocationSet):
            continue
        name = alloc.memorylocations[0].name
        if alloc.kind == "ExternalInput":
            if name != partition_name:
                in_names.append(name)
        elif alloc.kind == "ExternalOutput":
            out_names.append(name)
            shape = tuple(alloc.tensor_shape)
            dtype = mybir.dt.np(alloc.dtype)
            out_avals.append(jax.core.ShapedArray(shape, dtype))
            zero_outs.append(np.zeros(shape, dtype))
    n_params = len(in_names)
    all_names = in_names + out_names
    if partition_name is not None:
        all_names.append(partition_name)

    def _body(*args):
        operands = list(args)
        if partition_name is not None:
            operands.append(bass2jax.partition_id_tensor())
        return tuple(_bass_exec_p.bind(
            *operands,
            out_avals=tuple(out_avals),
            in_names=tuple(all_names),
            out_names=tuple(out_names),
            lowering_input_output_aliases=(),
            sim_require_finite=True,
            sim_require_nnan=True,
            nc=nc,
        ))

    devices = jax.devices()[:NCORES]
    mesh = Mesh(np.asarray(devices), ("core",))
    spec = NamedSharding(mesh, PartitionSpec("core"))
    nio = n_params + len(out_names)
    run = jax.jit(shard_map(
        _body, mesh=mesh, in_specs=(PartitionSpec("core"),) * nio,
        out_specs=(PartitionSpec("core"),) * len(out_names), check_rep=False),
        keep_unused=True)

    concat_in = [
        np.concatenate([np.asarray(in_maps[c][nm]) for c in range(NCORES)],
                       axis=0)
        for nm in in_names
    ]
    concat_zero = [np.zeros((NCORES * z.shape[0], *z.shape[1:]), z.dtype)
                   for z in zero_outs]
    dev_inputs = [jax.device_put(a, spec) for a in concat_in + concat_zero]
    return run, dev_inputs, out_names


# revision 15
# speedup vs baseline: 61.0829x; 1.2969x over previous
"""AttentionGNN (NNConv message-passing GNN) Trainium2 kernel.

Math restructuring: the reference materializes a per-edge [E,H,H] weight
tensor We = reshape(ef @ efW + efb) and does a per-edge GEMV + segment_sum.
We reorder the contraction:

    m[e]   = sum_k ef'[e,k] * (h[src_e] @ efWk)        (ef' = [ef, 1], efWk incl. bias block)
    agg[n] = sum_e[dst=n] m[e] / max(deg[n],1)
           = sum_k (Wk @ (h @ efWk))[n]

where Wk[dst,src] = sum_{e: dst,src} ef'[e,k] / max(deg[dst],1) is a
per-graph weighted adjacency matrix, built once on the host (edge features
do not change across layers).  The device kernel is then pure dense matmul:
no gather/scatter, no [E,H,H] tensor.

Sharding: data-parallel over graphs, 4 graphs per core on 8 cores,
parameters replicated.  Activations live transposed ([H, nodes]) so the
whole layer chains with zero on-device transposes.
"""

import numpy as np
import ml_dtypes

import concourse.bass as bass
import concourse.mybir as mybir
from concourse import bacc
from concourse.tile import TileContext

# ---- problem dimensions (hardcoded per spec) ----
B, L, R, H, EF = 32, 32, 128, 64, 16
NF_IN, EF_IN = 16, 4
LAYERS = 3
NCORES = 8
GPC = B // NCORES          # graphs per core = 4
NL = GPC * L               # lig nodes per core = 128
NR = GPC * R               # rec nodes per core = 512
KC = EF + 1                # 17 adjacency channels (16 edge feats + bias)
SLOPE = 0.01               # leaky_relu negative slope

# matmul operand dtype for the main chain: fp16 measured fastest
# (1 cyc/row on PE like bf16, 8x the mantissa) at 5.5e-4 rel err
import os
MM_DTYPE = os.environ.get("KMM_DTYPE", "f16")

_NP_D = {"bf16": ml_dtypes.bfloat16, "f16": np.float16, "f32": np.float32}
_MY_D = {"bf16": mybir.dt.bfloat16, "f16": mybir.dt.float16,
         "f32": mybir.dt.float32}

F32 = mybir.dt.float32
AF = mybir.ActivationFunctionType
ALU = mybir.AluOpType
AX = mybir.AxisListType

_CACHE = {}
LRELU_ACT = True   # use HW Lrelu activation (not implemented in CoreSim)


# --------------------------------------------------------------------------
# device kernel
# --------------------------------------------------------------------------

def _build_nc(dt_key, reps=1, loop_iters=1):
    D = _MY_D[dt_key]
    lrelu_act = LRELU_ACT
    nc = bacc.Bacc(None, target_bir_lowering=False)

    xl = nc.dram_tensor("xl", [NF_IN, NL], D, kind="ExternalInput")
    xr = nc.dram_tensor("xr", [NF_IN, NR], D, kind="ExternalInput")
    wl = nc.dram_tensor("wl", [NL, KC * NL], D, kind="ExternalInput")
    wr = nc.dram_tensor("wr", [R, GPC * KC * R], D, kind="ExternalInput")
    embw = nc.dram_tensor("embw", [NF_IN, 2 * H], D, kind="ExternalInput")
    hidw = nc.dram_tensor("hidw", [H, 2 * H], D, kind="ExternalInput")
    efwp = nc.dram_tensor("efwp", [H, 2 * LAYERS * KC * H], D, kind="ExternalInput")
    oww = nc.dram_tensor("oww", [H, 2 * LAYERS * H], D, kind="ExternalInput")
    bia = nc.dram_tensor("bia", [H, 16], F32, kind="ExternalInput")

    ops_o = nc.dram_tensor("ops_o", [GPC, L, R], F32, kind="ExternalOutput")
    sc_o = nc.dram_tensor("sc_o", [1, GPC], F32, kind="ExternalOutput")

    NALL = {0: NL, 1: NR}          # type 0 = lig, 1 = rec

    with TileContext(nc) as tc:
        with (
            tc.tile_pool(name="const", bufs=2 if reps * loop_iters > 1 else 1)
                as cpool,
            tc.tile_pool(name="work", bufs=3) as wpool,
            tc.tile_pool(name="hbars", bufs=3) as hpool,
            # PSUM budget (8 banks): Ga 2x2 + Gb 1 + agg0 + agg1 + h = 8
            tc.tile_pool(name="psG", bufs=2, space=bass.MemorySpace.PSUM) as psG,
            tc.tile_pool(name="psGb", bufs=1, space=bass.MemorySpace.PSUM) as psGb,
            tc.tile_pool(name="psA", bufs=1, space=bass.MemorySpace.PSUM) as psA,
            tc.tile_pool(name="psH", bufs=1, space=bass.MemorySpace.PSUM) as psH,
        ):
            def one_pass():
                # ---- load constants/params into SBUF ----
                # split across both HWDGE queues (SP + Activation)
                def load(dram, shape, dtype, tag, eng):
                    t = cpool.tile(shape, dtype, tag=tag)
                    eng.dma_start(out=t[:], in_=dram[:])
                    return t

                # small consts on the (otherwise idle) Pool SWDGE queue
                bia_sb = load(bia, [H, 16], F32, "bia", nc.gpsimd)
                xl_sb = load(xl, [NF_IN, NL], D, "xl", nc.gpsimd)
                xr_sb = load(xr, [NF_IN, NR], D, "xr", nc.gpsimd)
                embw_sb = load(embw, [NF_IN, 2 * H], D, "embw", nc.gpsimd)
                hidw_sb = load(hidw, [H, 2 * H], D, "hidw", nc.gpsimd)
                oww_sb = load(oww, [H, 2 * LAYERS * H], D, "oww", nc.gpsimd)
                # big blocks on the SP HWDGE queue, interleaved by need-time
                efwp_sb = cpool.tile([H, 2 * LAYERS * KC * H], D, tag="efwp")
                wl_sb = cpool.tile([NL, KC * NL], D, tag="wl")
                wr_sb = cpool.tile([R, GPC * KC * R], D, tag="wr")
                CW = 2 * KC * H
                def efwp_chunk(l):
                    nc.sync.dma_start(out=efwp_sb[:, l * CW:(l + 1) * CW],
                                      in_=efwp[:, l * CW:(l + 1) * CW])
                def wr_chunk(g):
                    nc.sync.dma_start(
                        out=wr_sb[:, g * KC * R:(g + 1) * KC * R],
                        in_=wr[:, g * KC * R:(g + 1) * KC * R])
                efwp_chunk(0)
                nc.sync.dma_start(out=wl_sb[:], in_=wl[:])
                wr_chunk(0)
                efwp_chunk(1)
                wr_chunk(1)
                efwp_chunk(2)
                wr_chunk(2)
                wr_chunk(3)
                ones_sb = cpool.tile([H, 1], F32, tag="ones")
                nc.gpsimd.memset(ones_sb[:], 1.0)

                def bcol(j):
                    return bia_sb[:, j:j + 1]

                # bias columns: 0 lig hid_b, 1 rec hid_b, 2 lig emb_nb,
                # 3 rec emb_nb, 4 + l*4 + t*2 + {0: gb, 1: ob}
                def stt_lrelu(out_ap, in_ap):
                    nc.vector.scalar_tensor_tensor(
                        out_ap, in_ap, SLOPE, in_ap, ALU.mult, ALU.max)

                def act_lrelu_bias(out_ap, psum_ap, bias_ap, t):
                    """out = leaky_relu(psum + bias), PSUM -> SBUF."""
                    if lrelu_act:
                        nc.scalar.activation(out_ap, psum_ap, AF.Lrelu,
                                             bias=bias_ap, alpha=SLOPE)
                    else:  # sim fallback (CoreSim lacks Lrelu)
                        n = psum_ap.shape[-1]
                        tmp = wpool.tile([H, n], F32, tag=f"xs{t}")
                        nc.scalar.activation(tmp[:], psum_ap, AF.Identity,
                                             bias=bias_ap)
                        stt_lrelu(out_ap, tmp[:])

                # ---- graph embedding: returns h1 PSUM (pending hid_b) ----
                def embed(t):
                    n = NALL[t]
                    x_sb = xl_sb if t == 0 else xr_sb
                    e1 = psA.tile([H, n], F32, tag=f"agg{t}")
                    nc.tensor.matmul(e1[:], embw_sb[:, t * H:(t + 1) * H],
                                     x_sb[:], start=True, stop=True)
                    ea = wpool.tile([H, n], D, tag=f"act2{t}")
                    act_lrelu_bias(ea[:], e1[:], bcol(2 + t), t)
                    h1 = psH.tile([H, n], F32, tag="h")
                    nc.tensor.matmul(h1[:], hidw_sb[:, t * H:(t + 1) * H],
                                     ea[:], start=True, stop=True)
                    return h1

                # ---- one MPNN layer for one graph type ----
                # src: ("psum", ht, bias_col_idx) or ("sbuf", hb)
                def layer(t, l, src, h0):
                    n = NALL[t]
                    t2 = l * 2 + t
                    act = wpool.tile([H, n], D, tag=f"act{t}")
                    if src[0] == "psum":
                        act_lrelu_bias(act[:], src[1][:], bcol(src[2]), t)
                    else:
                        stt_lrelu(act[:], src[1][:])

                    aggt = psA.tile([H, n], F32, tag=f"agg{t}")
                    ngr = 1 if t == 0 else GPC
                    base_c = t2 * KC * H

                    def lhs_g(g):
                        return act[:, g * R:(g + 1) * R] if t == 1 else act[:]

                    # bias-block G for all graphs up front: one bank, one copy
                    gball = psGb.tile([R, ngr * H], F32, tag="Gb")
                    for g in range(ngr):
                        nc.tensor.matmul(
                            gball[:, g * H:(g + 1) * H], lhs_g(g),
                            efwp_sb[:, base_c + 1024: base_c + KC * H],
                            start=True, stop=True)
                    gsb = wpool.tile([R, ngr * H], D, tag=f"gsb{t}")
                    nc.vector.tensor_copy(gsb[:], gball[:])

                    for g in range(ngr):
                        # G = act_g @ efW' (first 16 blocks) -> [128, 1024]
                        ga = psG.tile([R, 1024], F32, tag="Ga")
                        for c0 in (0, 512):
                            nc.tensor.matmul(
                                ga[:, c0:c0 + 512], lhs_g(g),
                                efwp_sb[:, base_c + c0: base_c + c0 + 512],
                                start=True, stop=True)
                        gs = wpool.tile([R, 1024], D,
                                        tag="Gs" if t == 1 else "Gsl")
                        nc.vector.tensor_copy(gs[:, 0:576], ga[:, 0:576])
                        nc.scalar.copy(gs[:, 576:1024], ga[:, 576:1024])
                        # aggT_g += G_k^T @ WkT, bias channel first
                        w_sb = wl_sb if t == 0 else wr_sb
                        base = 0 if t == 0 else g * KC * R
                        on = n if t == 0 else R
                        oc = 0 if t == 0 else g * R
                        nc.tensor.matmul(
                            aggt[:, oc:oc + on],
                            gsb[:, g * H:(g + 1) * H],
                            w_sb[:, base + (KC - 1) * on: base + KC * on],
                            start=True, stop=False)
                        for k in range(KC - 1):
                            nc.tensor.matmul(
                                aggt[:, oc:oc + on],
                                gs[:, k * H:(k + 1) * H],
                                w_sb[:, base + k * on: base + (k + 1) * on],
                                start=False, stop=(k == KC - 2))

                    act2 = wpool.tile([H, n], D, tag=f"act2{t}")
                    act_lrelu_bias(act2[:], aggt[:], bcol(4 + l * 4 + t * 2), t)
                    ht = psH.tile([H, n], F32, tag="h")
                    nc.tensor.matmul(ht[:], oww_sb[:, t2 * H:(t2 + 1) * H],
                                     act2[:], start=True, stop=True)
                    ob = 4 + l * 4 + t * 2 + 1
                    if l == 0:
                        # keep layer-0 output in SBUF f32: skip-conn source
                        hb = hpool.tile([H, n], F32, tag=f"hbar{t}")
                        nc.scalar.activation(hb[:], ht[:], AF.Identity,
                                             bias=bcol(ob))
                        return ("sbuf", hb), hb
                    if l == 1:
                        return ("psum", ht, ob), None
                    # l == 2: final = (ht + ob) + h0  in one DVE op
                    hb = hpool.tile([H, n], F32, tag=f"hbar{t}")
                    if t == 1:  # per-graph chunks let outer products start
                        for g in range(GPC):
                            s = slice(g * R, (g + 1) * R)
                            nc.vector.scalar_tensor_tensor(
                                hb[:, s], ht[:, s], bcol(ob), h0[:, s],
                                ALU.add, ALU.add)
                    else:
                        nc.vector.scalar_tensor_tensor(
                            hb[:], ht[:], bcol(ob), h0[:], ALU.add, ALU.add)
                    return ("sbuf", hb), None

                srcs = []
                for t in (0, 1):
                    srcs.append(("psum", embed(t), t))  # pending hid_b bias
                h0s = [None, None]
                for l in range(LAYERS):
                    for t in (0, 1):
                        srcs[t], maybe_h0 = layer(t, l, srcs[t], h0s[t])
                        if l == 0:
                            h0s[t] = maybe_h0
                finals = [srcs[0][1], srcs[1][1]]

                # ---- per-graph outer products + means (all fp32) ----
                hl, hr = finals
                sl = wpool.tile([H, GPC], F32, tag="sl")
                nc.vector.tensor_reduce(
                    sl[:], hl[:].rearrange("p (g n) -> p g n", g=GPC),
                    axis=AX.X, op=ALU.add)
                sr = wpool.tile([H, GPC], F32, tag="sr")
                nc.vector.tensor_reduce(
                    sr[:], hr[:].rearrange("p (g n) -> p g n", g=GPC),
                    axis=AX.X, op=ALU.add)
                pr = wpool.tile([H, GPC], F32, tag="pr")
                nc.vector.tensor_mul(pr[:], sl[:], sr[:])
                mm = psH.tile([1, GPC], F32, tag="h")
                nc.tensor.matmul(mm[:], ones_sb[:], pr[:],
                                 start=True, stop=True)
                sc = wpool.tile([1, GPC], F32, tag="sc")
                nc.scalar.mul(sc[:], mm[:], 1.0 / (L * R))
                nc.sync.dma_start(out=sc_o[:], in_=sc[:])
                opst = psA.tile([L, GPC * R], F32, tag="agg1")
                for g in range(GPC):
                    nc.tensor.matmul(opst[:, g * R:(g + 1) * R],
                                     hl[:, g * L:(g + 1) * L],
                                     hr[:, g * R:(g + 1) * R],
                                     start=True, stop=True)
                opss = wpool.tile([L, GPC * R], F32, tag="ops")
                nc.vector.tensor_copy(opss[:], opst[:])
                nc.sync.dma_start(
                    out=ops_o[:].rearrange("g l r -> l g r"),
                    in_=opss[:].rearrange("l (g r) -> l g r", g=GPC))


            if loop_iters > 1:
                with tc.For_i(0, loop_iters, 1):
                    for _rep in range(reps):
                        one_pass()
            else:
                for _rep in range(reps):
                    one_pass()

    nc.compile()
    return nc


def _get_nc(reps=1, loop_iters=1):
    key = ("nc", MM_DTYPE, reps, loop_iters, LRELU_ACT)
    if key not in _CACHE:
        _CACHE[key] = _build_nc(MM_DTYPE, reps, loop_iters)
    return _CACHE[key]


# --------------------------------------------------------------------------
# host-side preprocessing
# --------------------------------------------------------------------------

def _np(x):
    return np.asarray(x, dtype=np.float32)


def _build_adjacency(ef, src, dst, n_per):
    """Per-graph degree-normalized weighted adjacency, [B, src, dst, KC]."""
    src = np.asarray(src, dtype=np.int64)
    dst = np.asarray(dst, dtype=np.int64)
    e = ef.shape[0]
    deg = np.bincount(dst, minlength=B * n_per).astype(np.float32)
    scale = 1.0 / np.maximum(deg, 1.0)
    efp = np.concatenate([ef, np.ones((e, 1), np.float32)], axis=1)
    efp = efp * scale[dst][:, None]
    g = dst // n_per
    sl = src - g * n_per
    dl = dst - g * n_per
    ok = (sl >= 0) & (sl < n_per)   # edges never cross graphs per spec
    W = np.zeros((B * n_per * n_per, KC), np.float32)
    np.add.at(W, ((g * n_per + sl) * n_per + dl)[ok], efp[ok])
    return W.reshape(B, n_per, n_per, KC)


def _prepare_inputs(params, lig_x, lig_e, rec_x, rec_e,
                    lig_src, lig_dst, rec_src, rec_dst):
    p = params
    npd = _NP_D[MM_DTYPE]

    ef_l = _np(lig_e) @ _np(p["lig_emb_eW"]) + _np(p["lig_emb_eb"])
    ef_r = _np(rec_e) @ _np(p["rec_emb_eW"]) + _np(p["rec_emb_eb"])
    Wl = _build_adjacency(ef_l, lig_src, lig_dst, L)   # [B, s, d, KC]
    Wr = _build_adjacency(ef_r, rec_src, rec_dst, R)

    embw = np.concatenate([_np(p["lig_emb_nW"]), _np(p["rec_emb_nW"])], axis=1)
    hidw = np.concatenate([_np(p["lig_hid_W"]), _np(p["rec_hid_W"])], axis=1)

    efwp_blocks, oww_blocks = [], []
    bia = np.zeros((H, 16), np.float32)
    bia[:, 0] = _np(p["lig_hid_b"])
    bia[:, 1] = _np(p["rec_hid_b"])
    bia[:, 2] = _np(p["lig_emb_nb"])
    bia[:, 3] = _np(p["rec_emb_nb"])
    for l in range(LAYERS):
        lp = p["layers"][l]
        for t, pre in ((0, "lig_"), (1, "rec_")):
            efw = _np(lp[pre + "efW"]).reshape(EF, H, H).transpose(1, 0, 2)
            efb = _np(lp[pre + "efb"]).reshape(H, H)
            efwp_blocks.append(np.concatenate(
                [efw.reshape(H, EF * H), efb], axis=1))
            oww_blocks.append(_np(lp[pre + "oW"]))
            bia[:, 4 + l * 4 + t * 2] = _np(lp[pre + "gb"])
            bia[:, 4 + l * 4 + t * 2 + 1] = _np(lp[pre + "ob"])
    efwp = np.concatenate(efwp_blocks, axis=1)
    oww = np.concatenate(oww_blocks, axis=1)

    lig_x = _np(lig_x)
    rec_x = _np(rec_x)

    shared = {
        "embw": embw.astype(npd), "hidw": hidw.astype(npd),
        "efwp": efwp.astype(npd), "oww": oww.astype(npd),
        "bia": bia,
    }
    in_maps = []
    for c in range(NCORES):
        gs = slice(c * GPC, (c + 1) * GPC)
        # rec adjacency -> [src, (g_local, k, dst)] contiguous SBUF layout
        wr_c = Wr[gs].transpose(1, 0, 3, 2).reshape(R, GPC * KC * R)
        # lig adjacency -> block-diagonal over the 4 graphs: [128, KC*128]
        wl_c = np.zeros((NL, KC, NL), np.float32)
        Wl_c = Wl[gs]  # [GPC, s, d, KC]
        for gl in range(GPC):
            wl_c[gl * L:(gl + 1) * L, :, gl * L:(gl + 1) * L] = \
                Wl_c[gl].transpose(0, 2, 1)
        in_maps.append({
            "xl": lig_x[c * NL:(c + 1) * NL].T.astype(npd).copy(),
            "xr": rec_x[c * NR:(c + 1) * NR].T.astype(npd).copy(),
            "wl": wl_c.reshape(NL, KC * NL).astype(npd),
            "wr": wr_c.astype(npd),
            **shared,
        })
    return in_maps


def kernel(**inputs):
    nc = _get_nc()
    in_maps = _prepare_inputs(**inputs)
    from concourse.bass_utils import run_bass_kernel_spmd
    res = run_bass_kernel_spmd(nc, in_maps, core_ids=list(range(NCORES)))
    results = res.results
    ops = np.concatenate([r["ops_o"] for r in results], axis=0)
    out = np.concatenate([r["sc_o"].reshape(GPC) for r in results], axis=0)
    return out.astype(np.float32), ops.astype(np.float32)


# --------------------------------------------------------------------------
# reusable jitted runner (for benchmarking without re-compiles)
# --------------------------------------------------------------------------

def _get_runner(in_maps, reps=1, loop_iters=1):
    """Returns (run, dev_inputs): `run(*dev_inputs)` executes the kernel on
    all 8 cores and returns (ops_concat, sc_concat). Jit/compile happens once;
    repeated calls are pure dispatch+execute. No donation, so the same device
    buffers can be reused every call."""
    import jax
    from jax.sharding import Mesh, PartitionSpec, NamedSharding
    from jax.experimental.shard_map import shard_map
    from concourse import bass2jax
    from concourse.bass2jax import _bass_exec_p, install_neuronx_cc_hook

    nc = _get_nc(reps, loop_iters)
    install_neuronx_cc_hook()

    in_names, out_names, out_avals, zero_outs = [], [], [], []
    partition_name = (nc.partition_id_tensor.name
                      if nc.partition_id_tensor else None)
    for alloc in nc.m.functions[0].allocations:
        if not isinstance(alloc, mybir.MemoryLocationSet):
            continue
        name = alloc.memorylocations[0].name
        if alloc.kind == "ExternalInput":
            if name != partition_name:
                in_names.append(name)
        elif alloc.kind == "ExternalOutput":
            out_names.append(name)
            shape = tuple(alloc.tensor_shape)
            dtype = mybir.dt.np(alloc.dtype)
            out_avals.append(jax.core.ShapedArray(shape, dtype))
            zero_outs.append(np.zeros(shape, dtype))
    n_params = len(in_names)
    all_names = in_names + out_names
    if partition_name is not None:
        all_names.append(partition_name)

    def _body(*args):
        operands = list(args)
        if partition_name is not None:
            operands.append(bass2jax.partition_id_tensor())
        return tuple(_bass_exec_p.bind(
            *operands,
            out_avals=tuple(out_avals),
            in_names=tuple(all_names),
            out_names=tuple(out_names),
            lowering_input_output_aliases=(),
            sim_require_finite=True,
            sim_require_nnan=True,
            nc=nc,
        ))

    devices = jax.devices()[:NCORES]
    mesh = Mesh(np.asarray(devices), ("core",))
    spec = NamedSharding(mesh, PartitionSpec("core"))
    nio = n_params + len(out_names)
    run = jax.jit(shard_map(
        _body, mesh=mesh, in_specs=(PartitionSpec("core"),) * nio,
        out_specs=(PartitionSpec("core"),) * len(out_names), check_rep=False),
        keep_unused=True)

    concat_in = [
        np.concatenate([np.asarray(in_maps[c][nm]) for c in range(NCORES)],
                       axis=0)
        for nm in in_names
    ]
    concat_zero = [np.zeros((NCORES * z.shape[0], *z.shape[1:]), z.dtype)
                   for z in zero_outs]
    dev_inputs = [jax.device_put(a, spec) for a in concat_in + concat_zero]
    return run, dev_inputs, out_names


# revision 16
# speedup vs baseline: 66.8237x; 1.0940x over previous
"""AttentionGNN (NNConv message-passing GNN) Trainium2 kernel.

Math restructuring: the reference materializes a per-edge [E,H,H] weight
tensor We = reshape(ef @ efW + efb) and does a per-edge GEMV + segment_sum.
We reorder the contraction:

    m[e]   = sum_k ef'[e,k] * (h[src_e] @ efWk)        (ef' = [ef, 1], efWk incl. bias block)
    agg[n] = sum_e[dst=n] m[e] / max(deg[n],1)
           = sum_k (Wk @ (h @ efWk))[n]

where Wk[dst,src] = sum_{e: dst,src} ef'[e,k] / max(deg[dst],1) is a
per-graph weighted adjacency matrix, built once on the host (edge features
do not change across layers).  The device kernel is then pure dense matmul:
no gather/scatter, no [E,H,H] tensor.

Sharding: data-parallel over graphs, 4 graphs per core on 8 cores,
parameters replicated.  Activations live transposed ([H, nodes]) so the
whole layer chains with zero on-device transposes.
"""

import numpy as np
import ml_dtypes

import concourse.bass as bass
import concourse.mybir as mybir
from concourse import bacc
from concourse.tile import TileContext

# ---- problem dimensions (hardcoded per spec) ----
B, L, R, H, EF = 32, 32, 128, 64, 16
NF_IN, EF_IN = 16, 4
LAYERS = 3
NCORES = 8
GPC = B // NCORES          # graphs per core = 4
NL = GPC * L               # lig nodes per core = 128
NR = GPC * R               # rec nodes per core = 512
KC = EF + 1                # 17 adjacency channels (16 edge feats + bias)
SLOPE = 0.01               # leaky_relu negative slope

# matmul operand dtype for the main chain: fp16 measured fastest
# (1 cyc/row on PE like bf16, 8x the mantissa) at 5.5e-4 rel err
import os
MM_DTYPE = os.environ.get("KMM_DTYPE", "f16")

_NP_D = {"bf16": ml_dtypes.bfloat16, "f16": np.float16, "f32": np.float32}
_MY_D = {"bf16": mybir.dt.bfloat16, "f16": mybir.dt.float16,
         "f32": mybir.dt.float32}

F32 = mybir.dt.float32
AF = mybir.ActivationFunctionType
ALU = mybir.AluOpType
AX = mybir.AxisListType

_CACHE = {}
LRELU_ACT = True   # use HW Lrelu activation (not implemented in CoreSim)


# --------------------------------------------------------------------------
# device kernel
# --------------------------------------------------------------------------

def _build_nc(dt_key, reps=1, loop_iters=1):
    D = _MY_D[dt_key]
    lrelu_act = LRELU_ACT
    nc = bacc.Bacc(None, target_bir_lowering=False)

    xl = nc.dram_tensor("xl", [NF_IN, NL], D, kind="ExternalInput")
    xr = nc.dram_tensor("xr", [NF_IN, NR], D, kind="ExternalInput")
    wl = nc.dram_tensor("wl", [NL, KC * NL], D, kind="ExternalInput")
    wr = nc.dram_tensor("wr", [R, GPC * KC * R], D, kind="ExternalInput")
    embw = nc.dram_tensor("embw", [NF_IN, 2 * H], D, kind="ExternalInput")
    hidw = nc.dram_tensor("hidw", [H, 2 * H], D, kind="ExternalInput")
    efwp = nc.dram_tensor("efwp", [H, 2 * LAYERS * KC * H], D, kind="ExternalInput")
    oww = nc.dram_tensor("oww", [H, 2 * LAYERS * H], D, kind="ExternalInput")
    bia = nc.dram_tensor("bia", [H, 16], F32, kind="ExternalInput")

    ops_o = nc.dram_tensor("ops_o", [GPC, L, R], F32, kind="ExternalOutput")
    sc_o = nc.dram_tensor("sc_o", [1, GPC], F32, kind="ExternalOutput")

    NALL = {0: NL, 1: NR}          # type 0 = lig, 1 = rec

    with TileContext(nc) as tc:
        with (
            tc.tile_pool(name="const", bufs=2 if reps * loop_iters > 1 else 1)
                as cpool,
            tc.tile_pool(name="work", bufs=3) as wpool,
            tc.tile_pool(name="hbars", bufs=3) as hpool,
            # PSUM budget (8 banks): Ga 2x2 + Gb 1 + agg0 + agg1 + h = 8
            tc.tile_pool(name="psG", bufs=2, space=bass.MemorySpace.PSUM) as psG,
            tc.tile_pool(name="psGb", bufs=1, space=bass.MemorySpace.PSUM) as psGb,
            tc.tile_pool(name="psA", bufs=1, space=bass.MemorySpace.PSUM) as psA,
            tc.tile_pool(name="psH", bufs=1, space=bass.MemorySpace.PSUM) as psH,
        ):
            def one_pass():
                # ---- load constants/params into SBUF ----
                # split across both HWDGE queues (SP + Activation)
                def load(dram, shape, dtype, tag, eng):
                    t = cpool.tile(shape, dtype, tag=tag)
                    eng.dma_start(out=t[:], in_=dram[:])
                    return t

                # small consts on the (otherwise idle) Pool SWDGE queue
                bia_sb = load(bia, [H, 16], F32, "bia", nc.gpsimd)
                xl_sb = load(xl, [NF_IN, NL], D, "xl", nc.gpsimd)
                xr_sb = load(xr, [NF_IN, NR], D, "xr", nc.gpsimd)
                embw_sb = load(embw, [NF_IN, 2 * H], D, "embw", nc.gpsimd)
                hidw_sb = load(hidw, [H, 2 * H], D, "hidw", nc.gpsimd)
                oww_sb = load(oww, [H, 2 * LAYERS * H], D, "oww", nc.gpsimd)
                # big blocks on the SP HWDGE queue, interleaved by need-time
                efwp_sb = cpool.tile([H, 2 * LAYERS * KC * H], D, tag="efwp")
                wl_sb = cpool.tile([NL, KC * NL], D, tag="wl")
                wr_sb = cpool.tile([R, GPC * KC * R], D, tag="wr")
                CW = 2 * KC * H
                def efwp_chunk(l):
                    nc.sync.dma_start(out=efwp_sb[:, l * CW:(l + 1) * CW],
                                      in_=efwp[:, l * CW:(l + 1) * CW])
                def wr_chunk(g):
                    nc.sync.dma_start(
                        out=wr_sb[:, g * KC * R:(g + 1) * KC * R],
                        in_=wr[:, g * KC * R:(g + 1) * KC * R])
                efwp_chunk(0)
                nc.sync.dma_start(out=wl_sb[:], in_=wl[:])
                wr_chunk(0)
                efwp_chunk(1)
                wr_chunk(1)
                efwp_chunk(2)
                wr_chunk(2)
                wr_chunk(3)
                ones_sb = cpool.tile([H, 1], F32, tag="ones")
                nc.gpsimd.memset(ones_sb[:], 1.0)

                def bcol(j):
                    return bia_sb[:, j:j + 1]

                # bias columns: 0 lig hid_b, 1 rec hid_b, 2 lig emb_nb,
                # 3 rec emb_nb, 4 + l*4 + t*2 + {0: gb, 1: ob}
                def stt_lrelu(out_ap, in_ap):
                    nc.vector.scalar_tensor_tensor(
                        out_ap, in_ap, SLOPE, in_ap, ALU.mult, ALU.max)

                def act_lrelu_bias(out_ap, psum_ap, bias_ap, t):
                    """out = leaky_relu(psum + bias), PSUM -> SBUF."""
                    if lrelu_act:
                        nc.scalar.activation(out_ap, psum_ap, AF.Lrelu,
                                             bias=bias_ap, alpha=SLOPE)
                    else:  # sim fallback (CoreSim lacks Lrelu)
                        n = psum_ap.shape[-1]
                        tmp = wpool.tile([H, n], F32, tag=f"xs{t}")
                        nc.scalar.activation(tmp[:], psum_ap, AF.Identity,
                                             bias=bias_ap)
                        stt_lrelu(out_ap, tmp[:])

                # ---- graph embedding: returns h1 PSUM (pending hid_b) ----
                def embed(t):
                    n = NALL[t]
                    x_sb = xl_sb if t == 0 else xr_sb
                    e1 = psA.tile([H, n], F32, tag=f"agg{t}")
                    nc.tensor.matmul(e1[:], embw_sb[:, t * H:(t + 1) * H],
                                     x_sb[:], start=True, stop=True)
                    ea = wpool.tile([H, n], D, tag=f"act2{t}")
                    act_lrelu_bias(ea[:], e1[:], bcol(2 + t), t)
                    h1 = psH.tile([H, n], F32, tag="h")
                    nc.tensor.matmul(h1[:], hidw_sb[:, t * H:(t + 1) * H],
                                     ea[:], start=True, stop=True)
                    return h1

                # ---- one MPNN layer for one graph type ----
                # src: ("psum", ht, bias_col_idx) or ("sbuf", hb)
                def layer(t, l, src, h0):
                    n = NALL[t]
                    t2 = l * 2 + t
                    act = wpool.tile([H, n], D, tag=f"act{t}")
                    if src[0] == "psum":
                        act_lrelu_bias(act[:], src[1][:], bcol(src[2]), t)
                    else:
                        stt_lrelu(act[:], src[1][:])

                    aggt = psA.tile([H, n], F32, tag=f"agg{t}")
                    ngr = 1 if t == 0 else GPC
                    base_c = t2 * KC * H

                    def lhs_g(g):
                        return act[:, g * R:(g + 1) * R] if t == 1 else act[:]

                    # bias-block G for all graphs up front: one bank, one copy
                    gball = psGb.tile([R, ngr * H], F32, tag="Gb")
                    for g in range(ngr):
                        nc.tensor.matmul(
                            gball[:, g * H:(g + 1) * H], lhs_g(g),
                            efwp_sb[:, base_c + 1024: base_c + KC * H],
                            start=True, stop=True)
                    gsb = wpool.tile([R, ngr * H], D, tag=f"gsb{t}")
                    nc.vector.tensor_copy(gsb[:], gball[:])

                    for g in range(ngr):
                        # G = act_g @ efW' (first 16 blocks) -> [128, 1024]
                        ga = psG.tile([R, 1024], F32, tag="Ga")
                        for c0 in (0, 512):
                            nc.tensor.matmul(
                                ga[:, c0:c0 + 512], lhs_g(g),
                                efwp_sb[:, base_c + c0: base_c + c0 + 512],
                                start=True, stop=True)
                        gs = wpool.tile([R, 1024], D,
                                        tag="Gs" if t == 1 else "Gsl")
                        nc.vector.tensor_copy(gs[:, 0:576], ga[:, 0:576])
                        nc.scalar.copy(gs[:, 576:1024], ga[:, 576:1024])
                        # aggT_g += G_k^T @ WkT, bias channel first
                        w_sb = wl_sb if t == 0 else wr_sb
                        base = 0 if t == 0 else g * KC * R
                        on = n if t == 0 else R
                        oc = 0 if t == 0 else g * R
                        nc.tensor.matmul(
                            aggt[:, oc:oc + on],
                            gsb[:, g * H:(g + 1) * H],
                            w_sb[:, base + (KC - 1) * on: base + KC * on],
                            start=True, stop=False)
                        for k in range(KC - 1):
                            nc.tensor.matmul(
                                aggt[:, oc:oc + on],
                                gs[:, k * H:(k + 1) * H],
                                w_sb[:, base + k * on: base + (k + 1) * on],
                                start=False, stop=(k == KC - 2))

                    act2 = wpool.tile([H, n], D, tag=f"act2{t}")
                    act_lrelu_bias(act2[:], aggt[:], bcol(4 + l * 4 + t * 2), t)
                    ht = psH.tile([H, n], F32, tag="h")
                    nc.tensor.matmul(ht[:], oww_sb[:, t2 * H:(t2 + 1) * H],
                                     act2[:], start=True, stop=True)
                    ob = 4 + l * 4 + t * 2 + 1
                    if l == 0:
                        # keep layer-0 output in SBUF f32: skip-conn source
                        hb = hpool.tile([H, n], F32, tag=f"hbar{t}")
                        nc.scalar.activation(hb[:], ht[:], AF.Identity,
                                             bias=bcol(ob))
                        return ("sbuf", hb), hb
                    if l == 1:
                        return ("psum", ht, ob), None
                    # l == 2: final = (ht + ob) + h0  in one DVE op
                    hb = hpool.tile([H, n], F32, tag=f"hbar{t}")
                    if t == 1:  # per-graph chunks let outer products start
                        for g in range(GPC):
                            s = slice(g * R, (g + 1) * R)
                            nc.vector.scalar_tensor_tensor(
                                hb[:, s], ht[:, s], bcol(ob), h0[:, s],
                                ALU.add, ALU.add)
                    else:
                        nc.vector.scalar_tensor_tensor(
                            hb[:], ht[:], bcol(ob), h0[:], ALU.add, ALU.add)
                    return ("sbuf", hb), None

                srcs = []
                for t in (0, 1):
                    srcs.append(("psum", embed(t), t))  # pending hid_b bias
                h0s = [None, None]
                for l in range(LAYERS):
                    for t in (0, 1):
                        srcs[t], maybe_h0 = layer(t, l, srcs[t], h0s[t])
                        if l == 0:
                            h0s[t] = maybe_h0
                finals = [srcs[0][1], srcs[1][1]]

                # ---- per-graph outer products + means (all fp32) ----
                hl, hr = finals
                sl = wpool.tile([H, GPC], F32, tag="sl")
                nc.vector.tensor_reduce(
                    sl[:], hl[:].rearrange("p (g n) -> p g n", g=GPC),
                    axis=AX.X, op=ALU.add)
                sr = wpool.tile([H, GPC], F32, tag="sr")
                nc.vector.tensor_reduce(
                    sr[:], hr[:].rearrange("p (g n) -> p g n", g=GPC),
                    axis=AX.X, op=ALU.add)
                pr = wpool.tile([H, GPC], F32, tag="pr")
                nc.vector.tensor_mul(pr[:], sl[:], sr[:])
                mm = psH.tile([1, GPC], F32, tag="h")
                nc.tensor.matmul(mm[:], ones_sb[:], pr[:],
                                 start=True, stop=True)
                sc = wpool.tile([1, GPC], F32, tag="sc")
                nc.scalar.mul(sc[:], mm[:], 1.0 / (L * R))
                nc.sync.dma_start(out=sc_o[:], in_=sc[:])
                opst = psA.tile([L, GPC * R], F32, tag="agg1")
                for g in range(GPC):
                    nc.tensor.matmul(opst[:, g * R:(g + 1) * R],
                                     hl[:, g * L:(g + 1) * L],
                                     hr[:, g * R:(g + 1) * R],
                                     start=True, stop=True)
                opss = wpool.tile([L, GPC * R], F32, tag="ops")
                nc.vector.tensor_copy(opss[:], opst[:])
                nc.sync.dma_start(
                    out=ops_o[:].rearrange("g l r -> l g r"),
                    in_=opss[:].rearrange("l (g r) -> l g r", g=GPC))


            if loop_iters > 1:
                with tc.For_i(0, loop_iters, 1,
                              hint_engines=(mybir.EngineType.PE,)):
                    for _rep in range(reps):
                        one_pass()
            else:
                for _rep in range(reps):
                    one_pass()

    nc.compile()
    return nc


def _get_nc(reps=1, loop_iters=1):
    key = ("nc", MM_DTYPE, reps, loop_iters, LRELU_ACT)
    if key not in _CACHE:
        _CACHE[key] = _build_nc(MM_DTYPE, reps, loop_iters)
    return _CACHE[key]


# --------------------------------------------------------------------------
# host-side preprocessing
# --------------------------------------------------------------------------

def _np(x):
    return np.asarray(x, dtype=np.float32)


def _build_adjacency(ef, src, dst, n_per):
    """Per-graph degree-normalized weighted adjacency, [B, src, dst, KC]."""
    src = np.asarray(src, dtype=np.int64)
    dst = np.asarray(dst, dtype=np.int64)
    e = ef.shape[0]
    deg = np.bincount(dst, minlength=B * n_per).astype(np.float32)
    scale = 1.0 / np.maximum(deg, 1.0)
    efp = np.concatenate([ef, np.ones((e, 1), np.float32)], axis=1)
    efp = efp * scale[dst][:, None]
    g = dst // n_per
    sl = src - g * n_per
    dl = dst - g * n_per
    ok = (sl >= 0) & (sl < n_per)   # edges never cross graphs per spec
    W = np.zeros((B * n_per * n_per, KC), np.float32)
    np.add.at(W, ((g * n_per + sl) * n_per + dl)[ok], efp[ok])
    return W.reshape(B, n_per, n_per, KC)


def _prepare_inputs(params, lig_x, lig_e, rec_x, rec_e,
                    lig_src, lig_dst, rec_src, rec_dst):
    p = params
    npd = _NP_D[MM_DTYPE]

    ef_l = _np(lig_e) @ _np(p["lig_emb_eW"]) + _np(p["lig_emb_eb"])
    ef_r = _np(rec_e) @ _np(p["rec_emb_eW"]) + _np(p["rec_emb_eb"])
    Wl = _build_adjacency(ef_l, lig_src, lig_dst, L)   # [B, s, d, KC]
    Wr = _build_adjacency(ef_r, rec_src, rec_dst, R)

    embw = np.concatenate([_np(p["lig_emb_nW"]), _np(p["rec_emb_nW"])], axis=1)
    hidw = np.concatenate([_np(p["lig_hid_W"]), _np(p["rec_hid_W"])], axis=1)

    efwp_blocks, oww_blocks = [], []
    bia = np.zeros((H, 16), np.float32)
    bia[:, 0] = _np(p["lig_hid_b"])
    bia[:, 1] = _np(p["rec_hid_b"])
    bia[:, 2] = _np(p["lig_emb_nb"])
    bia[:, 3] = _np(p["rec_emb_nb"])
    for l in range(LAYERS):
        lp = p["layers"][l]
        for t, pre in ((0, "lig_"), (1, "rec_")):
            efw = _np(lp[pre + "efW"]).reshape(EF, H, H).transpose(1, 0, 2)
            efb = _np(lp[pre + "efb"]).reshape(H, H)
            efwp_blocks.append(np.concatenate(
                [efw.reshape(H, EF * H), efb], axis=1))
            oww_blocks.append(_np(lp[pre + "oW"]))
            bia[:, 4 + l * 4 + t * 2] = _np(lp[pre + "gb"])
            bia[:, 4 + l * 4 + t * 2 + 1] = _np(lp[pre + "ob"])
    efwp = np.concatenate(efwp_blocks, axis=1)
    oww = np.concatenate(oww_blocks, axis=1)

    lig_x = _np(lig_x)
    rec_x = _np(rec_x)

    shared = {
        "embw": embw.astype(npd), "hidw": hidw.astype(npd),
        "efwp": efwp.astype(npd), "oww": oww.astype(npd),
        "bia": bia,
    }
    in_maps = []
    for c in range(NCORES):
        gs = slice(c * GPC, (c + 1) * GPC)
        # rec adjacency -> [src, (g_local, k, dst)] contiguous SBUF layout
        wr_c = Wr[gs].transpose(1, 0, 3, 2).reshape(R, GPC * KC * R)
        # lig adjacency -> block-diagonal over the 4 graphs: [128, KC*128]
        wl_c = np.zeros((NL, KC, NL), np.float32)
        Wl_c = Wl[gs]  # [GPC, s, d, KC]
        for gl in range(GPC):
            wl_c[gl * L:(gl + 1) * L, :, gl * L:(gl + 1) * L] = \
                Wl_c[gl].transpose(0, 2, 1)
        in_maps.append({
            "xl": lig_x[c * NL:(c + 1) * NL].T.astype(npd).copy(),
            "xr": rec_x[c * NR:(c + 1) * NR].T.astype(npd).copy(),
            "wl": wl_c.reshape(NL, KC * NL).astype(npd),
            "wr": wr_c.astype(npd),
            **shared,
        })
    return in_maps


def kernel(**inputs):
    nc = _get_nc()
    in_maps = _prepare_inputs(**inputs)
    from concourse.bass_utils import run_bass_kernel_spmd
    res = run_bass_kernel_spmd(nc, in_maps, core_ids=list(range(NCORES)))
    results = res.results
    ops = np.concatenate([r["ops_o"] for r in results], axis=0)
    out = np.concatenate([r["sc_o"].reshape(GPC) for r in results], axis=0)
    return out.astype(np.float32), ops.astype(np.float32)


# --------------------------------------------------------------------------
# reusable jitted runner (for benchmarking without re-compiles)
# --------------------------------------------------------------------------

def _get_runner(in_maps, reps=1, loop_iters=1):
    """Returns (run, dev_inputs): `run(*dev_inputs)` executes the kernel on
    all 8 cores and returns (ops_concat, sc_concat). Jit/compile happens once;
    repeated calls are pure dispatch+execute. No donation, so the same device
    buffers can be reused every call."""
    import jax
    from jax.sharding import Mesh, PartitionSpec, NamedSharding
    from jax.experimental.shard_map import shard_map
    from concourse import bass2jax
    from concourse.bass2jax import _bass_exec_p, install_neuronx_cc_hook

    nc = _get_nc(reps, loop_iters)
    install_neuronx_cc_hook()

    in_names, out_names, out_avals, zero_outs = [], [], [], []
    partition_name = (nc.partition_id_tensor.name
                      if nc.partition_id_tensor else None)
    for alloc in nc.m.functions[0].allocations:
        if not isinstance(alloc, mybir.MemoryLocationSet):
            continue
        name = alloc.memorylocations[0].name
        if alloc.kind == "ExternalInput":
            if name != partition_name:
                in_names.append(name)
        elif alloc.kind == "ExternalOutput":
            out_names.append(name)
            shape = tuple(alloc.tensor_shape)
            dtype = mybir.dt.np(alloc.dtype)
            out_avals.append(jax.core.ShapedArray(shape, dtype))
            zero_outs.append(np.zeros(shape, dtype))
    n_params = len(in_names)
    all_names = in_names + out_names
    if partition_name is not None:
        all_names.append(partition_name)

    def _body(*args):
        operands = list(args)
        if partition_name is not None:
            operands.append(bass2jax.partition_id_tensor())
        return tuple(_bass_exec_p.bind(
            *operands,
            out_avals=tuple(out_avals),
            in_names=tuple(all_names),
            out_names=tuple(out_names),
            lowering_input_output_aliases=(),
            sim_require_finite=True,
            sim_require_nnan=True,
            nc=nc,
        ))

    devices = jax.devices()[:NCORES]
    mesh = Mesh(np.asarray(devices), ("core",))
    spec = NamedSharding(mesh, PartitionSpec("core"))
    nio = n_params + len(out_names)
    run = jax.jit(shard_map(
        _body, mesh=mesh, in_specs=(PartitionSpec("core"),) * nio,
        out_specs=(PartitionSpec("core"),) * len(out_names), check_rep=False),
        keep_unused=True)

    concat_in = [
        np.concatenate([np.asarray(in_maps[c][nm]) for c in range(NCORES)],
                       axis=0)
        for nm in in_names
    ]
    concat_zero = [np.zeros((NCORES * z.shape[0], *z.shape[1:]), z.dtype)
                   for z in zero_outs]
    dev_inputs = [jax.device_put(a, spec) for a in concat_in + concat_zero]
    return run, dev_inputs, out_names
